# revision 15
# baseline (speedup 1.0000x reference)
"""Trainium2 Bass kernel for nn_ABS_MHAtt (masked two-round multi-head attention).

Strategy: pure data-parallel over batch (B=16 -> 2 batches per NeuronCore, 8 cores,
no collectives). Host-side preprocessing (inside kernel()) pre-transposes
activations/weights into the [contraction, free] layouts the TensorEngine wants and
pre-converts everything to bf16, so the device kernel does zero layout conversion.

Per-core device kernel (per batch):
  - qhT/khT projections in transposed form [o, i]; v projected in natural form [j, o]
    directly into an "augmented" layout with a ones column per head (the ones column
    makes the PV/AV matmul also produce the softmax row-sum).
  - Per head: scores computed transposed [j, i] (contraction over d=64, head pairs
    row-tiled onto the two PE array halves), exp on ScalarE, masking by multiplying
    with (1-mask)^T (split across VectorE and GpSimdE), PV/AV with E as the
    stationary operand, one fused broadcast-multiply normalize per head, and DMA
    xbar transposes (not TensorE) to repack [i, d] tiles back to [d, i].

v3 scheduling: single interleaved stage stream. Prologue = batch 0's three
projections (dense PE while inputs land). Then 8 "duos" of attention pairs mixing
BOTH batches -- (b0p0,b0p1), (b0p2,b1p0), ..., (b1p6,b1p7) -- each duo's 8 stage
slots followed by one ~0.9us filler thunk: batch 1's projections fill the early
duos, batch 0's output projection fills the late duos, so the PE never idles long
enough for the HAM clock gate to re-throttle (attention alone is ScalarE-exp-bound
at ~8.6us/pair vs ~5.1us of PE work). imt/aug are streamed per-pair through small
rotating pools (2-duo lookahead) so both batches' working sets fit in SBUF.
ScalarE runs ONLY exp activations (plus prologue-phase evacuations and tail
stores); bulk loads ride Sync, per-pair loads ride GpSimd.
"""

import os
import sys

import numpy as np


def _ensure_concourse():
    try:
        import concourse.bass  # noqa: F401
        return
    except Exception:
        pass
    for p in ("/opt/trn_rl_repo", "/root/.axon_site/_ro/trn_rl_repo"):
        if os.path.isdir(p) and p not in sys.path:
            sys.path.insert(0, p)
            try:
                import concourse.bass  # noqa: F401
                return
            except Exception:
                sys.path.remove(p)
    raise ImportError("cannot import concourse (bass)")


B, L, HS = 16, 512, 1024
H, D = 16, 64
NCORES = 8
BPC = B // NCORES  # batches per core
SCALE = 1.0 / 8.0  # 1/sqrt(D)
AUGW = 65  # per-head augmented width (D + ones column)

_CACHE = {}


def _build_nc():
    _ensure_concourse()
    import concourse.bass as bass  # noqa: F401
    import concourse.mybir as mybir
    import concourse.tile as tile
    from concourse import bacc
    from contextlib import ExitStack

    bf = mybir.dt.bfloat16
    f32 = mybir.dt.float32
    Exp = mybir.ActivationFunctionType.Exp

    nc = bacc.Bacc()

    # all inputs host-preswizzled to [128, free] per-partition-contiguous
    # layouts so every load is one cheap 2D DMA
    qt = nc.declare_dram_parameter("qt", [BPC, 128, 8 * L], bf, isOutput=False)
    kt = nc.declare_dram_parameter("kt", [BPC, 128, 8 * L], bf, isOutput=False)
    vt = nc.declare_dram_parameter("vt", [BPC, 128, 8 * L], bf, isOutput=False)
    imt = nc.declare_dram_parameter("imt", [BPC, 128, 8 * L], bf, isOutput=False)
    aug = nc.declare_dram_parameter(
        "aug", [BPC, 128, 4 * H * AUGW], bf, isOutput=False
    )
    kp1 = nc.declare_dram_parameter("kp1", [BPC, 128, 4 * L], bf, isOutput=False)
    kp2 = nc.declare_dram_parameter("kp2", [BPC, 128, 4 * L], bf, isOutput=False)
    wq = nc.declare_dram_parameter("wq", [128, 8 * HS], bf, isOutput=False)
    wk = nc.declare_dram_parameter("wk", [128, 8 * HS], bf, isOutput=False)
    wv = nc.declare_dram_parameter("wv", [128, 8 * HS], bf, isOutput=False)
    wm = nc.declare_dram_parameter("wm", [128, 8 * HS], bf, isOutput=False)
    idt = nc.declare_dram_parameter("idt", [128, 128], bf, isOutput=False)
    out = nc.declare_dram_parameter("out", [BPC, L, HS], bf, isOutput=True)

    with ExitStack() as ctx:
        tc = ctx.enter_context(tile.TileContext(nc))
        consts = ctx.enter_context(tc.tile_pool(name="consts", bufs=1))
        inp = ctx.enter_context(tc.tile_pool(name="inp", bufs=1))
        rot = ctx.enter_context(tc.tile_pool(name="rot", bufs=6))
        proj = ctx.enter_context(tc.tile_pool(name="proj", bufs=2))
        ework = ctx.enter_context(tc.tile_pool(name="ework", bufs=2))
        small = ctx.enter_context(tc.tile_pool(name="small", bufs=3))
        evac = ctx.enter_context(tc.tile_pool(name="evac", bufs=2))
        psA = ctx.enter_context(tc.tile_pool(name="psA", bufs=3, space="PSUM"))
        psT = ctx.enter_context(tc.tile_pool(name="psT", bufs=2, space="PSUM"))

        w_sb = {}

        def load_weight_half(name, wext, half, tag=None):
            tag = tag or name
            if name not in w_sb:
                t = consts.tile([128, 8, HS], bf, tag=tag, name=name + "_sb")
                w_sb[name] = t
            t = w_sb[name]
            nc.sync.dma_start(
                out=t[:, half * 4 : (half + 1) * 4, :],
                in_=wext[:, half * 4 * HS : (half + 1) * 4 * HS],
            )

        # ---- bulk per-batch input tiles (qt/kt/vt shared between batches:
        # batch 1's load WAR-waits on batch 0's last projection reader) ----
        xin = {0: {}, 1: {}}

        def load_x_half(t, ext, b, half):
            # activation bulk loads ride the ScalarE HWDGE queue (exp-idle
            # during the prologue), in parallel with the weight stream on Sync
            nc.scalar.dma_start(
                out=t[:, half * 4 : (half + 1) * 4, :],
                in_=ext[b][:, half * 4 * L : (half + 1) * 4 * L],
            )

        def load_bulk(b, which):
            ext = {"qt": qt, "kt": kt, "vt": vt}[which]
            t = inp.tile([128, 8, L], bf, tag=which, name=which + "_sb")
            for half in range(2):
                load_x_half(t, ext, b, half)
            xin[b][which] = t

        def load_vt_jt(b):
            # jt-sliced vt load: 4 DMAs of [128, 8, 128] so the first vaug
            # group's slice lands after ~0.25MB instead of 1MB
            t = inp.tile([128, 8, L], bf, tag="vt", name="vt_sb")
            ext = vt[b].rearrange("p (c x) -> p c x", x=L)
            for jt in range(4):
                nc.scalar.dma_start(
                    out=t[:, :, jt * 128 : (jt + 1) * 128],
                    in_=ext[:, :, jt * 128 : (jt + 1) * 128],
                )
            xin[b]["vt"] = t

        def load_kp(b, which):
            ext = {"kp1": kp1, "kp2": kp2}[which]
            t = inp.tile([128, 4, L], bf, tag=f"{which}_{b}", name=which + "_sb")
            nc.gpsimd.dma_start(out=t, in_=ext[b])
            xin[b][which] = t

        # ---- per-pair rotating imt/aug tiles (2-duo lookahead) ----
        pair_imt = {}
        pair_aug = {}

        def load_pair(b, hp):
            ti = rot.tile([128, L], bf, tag="imt", name="imt_p")
            nc.gpsimd.dma_start(
                out=ti, in_=imt[b][:, hp * L : (hp + 1) * L]
            )
            pair_imt[(b, hp)] = ti
            ta = rot.tile([128, 4, 2 * AUGW], bf, tag="aug", name="aug_p")
            nc.gpsimd.dma_start(
                out=ta,
                in_=aug[b].rearrange("p (jt x) -> p jt x", x=H * AUGW)[
                    :, :, 2 * hp * AUGW : (2 * hp + 2) * AUGW
                ],
            )
            pair_aug[(b, hp)] = ta

        # ---- startup: wq/qt in fine-grained interleaved chunks so the first
        # projection matmuls can start after ~400KB instead of 3MB ----
        ident = consts.tile([128, 128], bf, tag="ident")
        xin[0]["qt"] = inp.tile([128, 8, L], bf, tag="qt", name="qt_sb")
        wq_t = consts.tile([128, 8, HS], bf, tag="wq", name="wq_sb")
        w_sb["wq"] = wq_t
        for c in range(4):
            nc.sync.dma_start(
                out=wq_t[:, 2 * c : 2 * c + 2, :],
                in_=wq[:, 2 * c * HS : (2 * c + 2) * HS],
            )
            nc.scalar.dma_start(
                out=xin[0]["qt"][:, 2 * c : 2 * c + 2, :],
                in_=qt[0][:, 2 * c * L : (2 * c + 2) * L],
            )
        nc.gpsimd.dma_start(out=ident, in_=idt[:, :])

        # ---- per-batch working tiles ----
        st = {}
        for b in range(BPC):
            st[b] = {
                "qh": proj.tile([128, 8, L], bf, tag="qh", name="qh_sb"),
                "kh": proj.tile([128, 8, L], bf, tag="kh", name="kh_sb"),
                "vaug": proj.tile(
                    [128, 4, H * AUGW], bf, tag="vaug", name="vaug_sb"
                ),
                "att": proj.tile([128, 8, L], bf, tag="att", name="att_sb"),
            }

        # ---- projection groups (one PSUM group each; interleavable thunks) ----
        def proj_qk_part(b, wname, dstname, ot, part, state):
            """half of a projection PSUM group (~0.9us of PE work)."""
            wt = w_sb[wname]
            xsb = xin[b]["qt" if wname == "wq" else "kt"]
            dst = st[b][dstname]
            if part == 0:
                state["ps"] = psA.tile([128, 512], f32, tag="psA", name="ps_pj")
            ps = state["ps"]
            for kc in range(4 * part, 4 * part + 4):
                nc.tensor.matmul(
                    ps,
                    wt[:, kc, ot * 128 : (ot + 1) * 128],
                    xsb[:, kc, :],
                    start=(kc == 0),
                    stop=(kc == 7),
                )
            if part == 1:
                if b == 0:
                    # b0's evacs ride ScalarE (exp-idle during the prologue);
                    # b1's (issued while ScalarE is dense with exp) ride DVE
                    nc.scalar.copy(out=dst[:, ot, :], in_=ps)
                else:
                    nc.vector.tensor_copy(out=dst[:, ot, :], in_=ps)

        def proj_qk_group(b, wname, dstname, ot):
            state = {}
            proj_qk_part(b, wname, dstname, ot, 0, state)
            proj_qk_part(b, wname, dstname, ot, 1, state)

        def vaug_part(b, jt, oh, part, state):
            """half of a v-projection group into the augmented layout."""
            vaug_sb = st[b]["vaug"]
            vt_sb = xin[b]["vt"]
            if part == 0 and oh == 0:
                nc.vector.memset(
                    vaug_sb[:, jt, :].rearrange("p (h x) -> p h x", x=AUGW)[
                        :, :, 64
                    ],
                    1.0,
                )
            if part == 0:
                state["ps"] = psA.tile([128, 512], f32, tag="psA", name="ps_pj")
            ps = state["ps"]
            for kc in range(4 * part, 4 * part + 4):
                nc.tensor.matmul(
                    ps,
                    vt_sb[:, kc, jt * 128 : (jt + 1) * 128],
                    w_sb["wv"][:, kc, oh * 512 : (oh + 1) * 512],
                    start=(kc == 0),
                    stop=(kc == 7),
                )
            if part == 1:
                dst_ap = vaug_sb[
                    :, jt, oh * 8 * AUGW : (oh + 1) * 8 * AUGW
                ].rearrange("p (h x) -> p h x", x=AUGW)[:, :, 0:64]
                nc.vector.tensor_copy(
                    out=dst_ap, in_=ps.rearrange("p (h x) -> p h x", x=64)
                )

        def vaug_group(b, jt, oh):
            state = {}
            vaug_part(b, jt, oh, 0, state)
            vaug_part(b, jt, oh, 1, state)

        def outproj_part(b, it, oh, part, state):
            """half of an output-projection PSUM group (~0.9us of PE work)."""
            att_sb = st[b]["att"]
            if part == 0:
                state["ps"] = psA.tile([128, 512], f32, tag="psA", name="ps_pj")
            ps = state["ps"]
            for kc in range(4 * part, 4 * part + 4):
                nc.tensor.matmul(
                    ps,
                    att_sb[:, kc, it * 128 : (it + 1) * 128],
                    w_sb["wm"][:, kc, oh * 512 : (oh + 1) * 512],
                    start=(kc == 0),
                    stop=(kc == 7),
                )
            if part == 1:
                ob = evac.tile([128, 512], bf, tag="ob")
                nc.vector.tensor_copy(out=ob, in_=ps)
                # batch 1's stores run in the kernel tail where ScalarE is
                # idle; batch 0's ride Sync during the attention region
                eng = nc.scalar if b == 1 else nc.sync
                eng.dma_start(
                    out=out[
                        b, it * 128 : (it + 1) * 128, oh * 512 : (oh + 1) * 512
                    ],
                    in_=ob,
                )

        def outproj_group(b, it, oh):
            state = {}
            outproj_part(b, it, oh, 0, state)
            outproj_part(b, it, oh, 1, state)

        # ---- attention stages ----
        def score_stage(b, hp, lhs_fn, rhs_fn, etile):
            """s^T [j,i] for both heads of pair hp + exp into etile.

            The two heads' matmuls use lhsT base partitions 0 / 64, so they
            run concurrently on the two row-halves of the PE array (outputs
            land in different PSUM banks)."""
            for jt in range(4):
                ps = psA.tile([128, 1024], f32, tag="psA", name="ps_sc")
                for g in range(2):
                    nc.tensor.matmul(
                        ps[:, g * 512 : (g + 1) * 512],
                        lhs_fn(g, jt),
                        rhs_fn(g),
                        start=True,
                        stop=True,
                    )
                nc.scalar.activation(
                    out=etile[:, jt],
                    in_=ps.rearrange("p (g x) -> p g x", x=512),
                    func=Exp,
                    scale=SCALE,
                )

        def mask_stage(b, hp, kp_sb, etile):
            # in-place mask multiply, one fused op per jt covering both heads
            # (mask row broadcast across the head dim via a stride-0 AP).
            # One jt per odd pair on GpSimd to offload DVE; its ~2.2us op
            # latency hides behind the neighbouring stages' matmuls.
            for jt in range(4):
                kpb = kp_sb[:, jt, :].unsqueeze(1).broadcast_to([128, 2, L])
                eng = nc.gpsimd if (jt == 3 and hp % 2 == 1) else nc.vector
                eng.tensor_mul(etile[:, jt], etile[:, jt], kpb)

        def pv_stage(b, hp, emtile, rhs_fn):
            """pv natural [i, 4*65] per head -> normalized dl pair [128,4,128].

            One fused broadcast-multiply per head turns the raw PSUM pv tile
            into the normalized bf16 dl tile (recip row-sums broadcast along
            d via a stride-0 AP)."""
            dl = small.tile([128, 4, 128], bf, tag="dl")
            for g in range(2):
                pspv = psT.tile([128, 4, AUGW], f32, tag="tail")
                for it in range(4):
                    for jt in range(4):
                        nc.tensor.matmul(
                            pspv[:, it, :],
                            emtile[:, jt, g, it * 128 : (it + 1) * 128],
                            rhs_fn(g, jt),
                            start=(jt == 0),
                            stop=(jt == 3),
                        )
                r1 = small.tile([128, 4], f32, tag="r1")
                nc.vector.reciprocal(r1, pspv[:, :, 64])
                nc.vector.tensor_mul(
                    dl[:, :, g * 64 : (g + 1) * 64],
                    pspv[:, :, 0:64],
                    r1.unsqueeze(-1).broadcast_to([128, 4, 64]),
                )
            return dl

        def s1_stage(b, hp):
            e1 = ework.tile([128, 4, 2, L], bf, tag="e")
            imt_t = pair_imt[(b, hp)]
            qh_sb = st[b]["qh"]
            score_stage(
                b,
                hp,
                lambda g, jt: imt_t[g * 64 : g * 64 + 64, jt * 128 : (jt + 1) * 128],
                lambda g: qh_sb[g * 64 : g * 64 + 64, hp, :],
                e1,
            )
            mask_stage(b, hp, xin[b]["kp1"], e1)
            return e1

        def mod_stage(b, hp, e1):
            """round-1 tail: pv + normalize + PE-transpose + add qh -> qn_pair.

            This transpose is on the s2 latency chain, so it stays on TensorE
            instead of the ~1.2us-per-op DMA xbar path."""
            aug_t = pair_aug[(b, hp)]
            dl = pv_stage(
                b, hp, e1,
                lambda g, jt: aug_t[:, jt, g * AUGW : (g + 1) * AUGW],
            )
            pst = psT.tile([128, 512], bf, tag="tail", name="pst")
            for it in range(4):
                nc.tensor.transpose(
                    pst[:, it * 128 : (it + 1) * 128], dl[:, it, :], ident
                )
            qn_pair = small.tile([128, 512], bf, tag="qnp")
            nc.vector.tensor_add(qn_pair, pst, st[b]["qh"][:, hp, :])
            return qn_pair

        def s2_stage(b, hp, qn_pair):
            e2 = ework.tile([128, 4, 2, L], bf, tag="e")
            kh_sb = st[b]["kh"]
            score_stage(
                b,
                hp,
                lambda g, jt: kh_sb[g * 64 : g * 64 + 64, hp, jt * 128 : (jt + 1) * 128],
                lambda g: qn_pair[g * 64 : g * 64 + 64, :],
                e2,
            )
            mask_stage(b, hp, xin[b]["kp2"], e2)
            return e2

        def av_stage(b, hp, e2):
            """round-2 tail: av + normalize + DMA-transpose -> att[:, hp, :]."""
            vaug_sb = st[b]["vaug"]
            dl = pv_stage(
                b, hp, e2,
                lambda g, jt: vaug_sb[:, jt, (2 * hp + g) * AUGW : (2 * hp + g + 1) * AUGW],
            )
            for it in range(4):
                nc.sync.dma_start_transpose(
                    out=st[b]["att"][:, hp, it * 128 : (it + 1) * 128],
                    in_=dl[:, it, :],
                )

        # ---- stage-stream runner ----
        def do_s1(p):
            p["e1"] = s1_stage(p["b"], p["hp"])

        def do_mod(p):
            p["qn"] = mod_stage(p["b"], p["hp"], p["e1"])

        def do_s2(p):
            p["e2"] = s2_stage(p["b"], p["hp"], p["qn"])

        def do_av(p):
            av_stage(p["b"], p["hp"], p["e2"])

        def run_duos(duos, fillers, preduo_hooks):
            for di, (pa, pb) in enumerate(duos):
                for hook in preduo_hooks.get(di, ()):
                    hook()
                for fn, p in (
                    (do_s1, pa), (do_s1, pb),
                    (do_mod, pa), (do_mod, pb),
                    (do_s2, pa), (do_s2, pb),
                    (do_av, pa), (do_av, pb),
                ):
                    if p is not None:
                        fn(p)
                    if fillers:
                        t = fillers.pop(0)
                        if t is not None:
                            t()
            while fillers:
                t = fillers.pop(0)
                if t is not None:
                    t()

        # ---- schedule ----
        # prologue: batch 0's three projections, dense PE while inputs land.
        # qh first (so qt(b1)'s WAR clears early), then kh, then vaug.
        # Weight stream on Sync; activation bulk loads on DVE's queue
        # (parallel DMA queues halve the prologue's DMA critical path);
        # per-pair imt/aug + keep-masks on GpSimd's queue.
        load_kp(0, "kp1")
        load_kp(0, "kp2")
        # imt/aug for duo0+duo1's pairs (hooks cover duo2 onward)
        for p_ in ((0, 0), (0, 1), (0, 2), (1, 0)):
            load_pair(*p_)
        kt0 = inp.tile([128, 8, L], bf, tag="kt", name="kt_sb")
        xin[0]["kt"] = kt0
        vt0 = inp.tile([128, 8, L], bf, tag="vt", name="vt_sb")
        xin[0]["vt"] = vt0
        for ot in range(8):
            if ot < 2:
                load_weight_half("wk", wk, ot)
                load_x_half(kt0, kt, 0, ot)
            elif ot < 4:
                load_weight_half("wv", wv, ot - 2)
                load_x_half(vt0, vt, 0, ot - 2)
            elif ot < 6:
                load_weight_half("wm", wm, ot - 4)
            proj_qk_group(0, "wq", "qh", ot)
        load_bulk(1, "qt")  # WAR on b0's qh groups just cleared
        load_kp(1, "kp1")
        for ot in range(8):
            proj_qk_group(0, "wk", "kh", ot)
        load_bulk(1, "kt")
        load_kp(1, "kp2")
        for jt in range(4):
            for oh in range(2):
                vaug_group(0, jt, oh)
        load_vt_jt(1)

        P = {}
        for b in range(2):
            for hp in range(8):
                P[(b, hp)] = {"b": b, "hp": hp}

        duos = [
            (P[(0, 0)], P[(0, 1)]),
            (P[(0, 2)], P[(1, 0)]),
            (P[(0, 3)], P[(1, 1)]),
            (P[(0, 4)], P[(1, 2)]),
            (P[(0, 5)], P[(1, 3)]),
            (P[(0, 6)], P[(0, 7)]),
            (P[(1, 4)], P[(1, 5)]),
            (P[(1, 6)], P[(1, 7)]),
        ]

        def halves(fn, *args):
            state = {}
            return [
                (lambda part=part, state=state: fn(*args, part, state))
                for part in range(2)
            ]

        fillers = []
        # duo0: b1's first qh/kh groups + first half of vaug-oh0
        fillers += halves(proj_qk_part, 1, "wq", "qh", 0)
        fillers += halves(proj_qk_part, 1, "wk", "kh", 0)
        fillers += halves(vaug_part, 1, 0, 0)
        fillers += halves(vaug_part, 1, 1, 0)
        # duo1: rest of vaug-oh0 (b1p0's av needs all jt), then ot1
        fillers += halves(vaug_part, 1, 2, 0)
        fillers += halves(vaug_part, 1, 3, 0)
        fillers += halves(proj_qk_part, 1, "wq", "qh", 1)
        fillers += halves(proj_qk_part, 1, "wk", "kh", 1)
        # duo2-4: qh/kh ot2..7
        for ot in range(2, 8):
            fillers += halves(proj_qk_part, 1, "wq", "qh", ot)
            fillers += halves(proj_qk_part, 1, "wk", "kh", ot)
        # duo5: vaug-oh1 (b1p4's av in duo6 needs it)
        for jt in range(4):
            fillers += halves(vaug_part, 1, jt, 1)
        # duo6-7: b0's output projection (b0p7's av ends in duo5)
        for it in range(4):
            for oh in range(2):
                fillers += halves(outproj_part, 0, it, oh)

        hooks = {}
        for di in range(len(duos)):
            hs = []
            if di + 2 < len(duos):
                for p in duos[di + 2]:
                    if p is not None:
                        hs.append(
                            lambda b=p["b"], hp=p["hp"]: load_pair(b, hp)
                        )
            hooks[di] = hs

        run_duos(duos, fillers, hooks)

        # tail: batch 1's output projection (ScalarE idle -> stores on scalar)
        for it in range(4):
            for oh in range(2):
                outproj_group(1, it, oh)

    nc.compile()
    return nc


def _get_nc():
    if "nc" not in _CACHE:
        _CACHE["nc"] = _build_nc()
    return _CACHE["nc"]


def _prep_inputs(v, k, q, img_abs, Wv, Wk, Wq, Wm, abs_mask, mask):
    import ml_dtypes

    bf16 = ml_dtypes.bfloat16
    f32 = np.float32

    def swz(x, nt):  # [B, nt*128, F] -> [B, 128, nt*F] partition-contiguous
        b, r, f = x.shape
        return np.ascontiguousarray(
            x.reshape(b, nt, 128, f).transpose(0, 2, 1, 3).reshape(b, 128, nt * f)
        )

    def t_bf(x):  # [B, L, HS] -> [B, 128, 8*L] bf16 swizzled
        xt = np.swapaxes(np.asarray(x, f32), 1, 2)
        return swz(xt, 8).astype(bf16)

    qt = t_bf(q)
    ktr = t_bf(k)
    vtr = t_bf(v)
    imt = t_bf(img_abs)

    img = np.asarray(img_abs, f32)
    augf = np.empty((B, L, H * AUGW), f32)
    augf.reshape(B, L, H, AUGW)[..., :64] = img.reshape(B, L, H, 64)
    augf.reshape(B, L, H, AUGW)[..., 64] = 1.0
    augv = swz(augf, 4).astype(bf16)

    def keepT(m):  # [B, 1, L, L] bool -> (1-m)^T swizzled bf16
        kf = 1.0 - np.asarray(m, f32)[:, 0]
        return swz(np.swapaxes(kf, 1, 2), 4).astype(bf16)

    kp1 = keepT(abs_mask)
    kp2 = keepT(mask)

    def wT(w):
        wt = np.asarray(w, f32).T  # [i, o]
        return swz(wt[None], 8)[0].astype(bf16)

    wqs, wks, wvs, wms = wT(Wq), wT(Wk), wT(Wv), wT(Wm)
    ident = np.eye(128, dtype=bf16)

    in_maps = []
    for c in range(NCORES):
        s = slice(c * BPC, (c + 1) * BPC)
        in_maps.append(
            {
                "qt": qt[s],
                "kt": ktr[s],
                "vt": vtr[s],
                "imt": imt[s],
                "aug": augv[s],
                "kp1": kp1[s],
                "kp2": kp2[s],
                "wq": wqs,
                "wk": wks,
                "wv": wvs,
                "wm": wms,
                "idt": ident,
            }
        )
    return in_maps


def kernel(v, k, q, img_abs, Wv, Wk, Wq, Wm, abs_mask, mask, _trace=False):
    _ensure_concourse()
    from concourse.bass_utils import run_bass_kernel_spmd

    in_maps = _prep_inputs(v, k, q, img_abs, Wv, Wk, Wq, Wm, abs_mask, mask)
    nc = _get_nc()
    res = run_bass_kernel_spmd(nc, in_maps, core_ids=list(range(NCORES)), trace=_trace)
    outp = np.concatenate([res.results[i]["out"] for i in range(NCORES)], axis=0)
    outp = np.asarray(outp, np.float32)  # device stores bf16; upcast on host
    if _trace:
        _CACHE["last_result"] = res
    return outp


# revision 16
# speedup vs baseline: 1.0896x; 1.0896x over previous
"""Trainium2 Bass kernel for nn_ABS_MHAtt (masked two-round multi-head attention).

Strategy: pure data-parallel over batch (B=16 -> 2 batches per NeuronCore, 8 cores,
no collectives). Host-side preprocessing (inside kernel()) pre-transposes
activations/weights into the [contraction, free] layouts the TensorEngine wants and
pre-converts everything to bf16, so the device kernel does zero layout conversion.

Per-core device kernel (per batch):
  - qhT/khT projections in transposed form [o, i]; v projected in natural form [j, o]
    directly into an "augmented" layout with a ones column per head (the ones column
    makes the PV/AV matmul also produce the softmax row-sum).
  - Per head: scores computed transposed [j, i] (contraction over d=64, head pairs
    row-tiled onto the two PE array halves), exp on ScalarE, masking by multiplying
    with (1-mask)^T (split across VectorE and GpSimdE), PV/AV with E as the
    stationary operand, one fused broadcast-multiply normalize per head, and DMA
    xbar transposes (not TensorE) to repack [i, d] tiles back to [d, i].

v3 scheduling: single interleaved stage stream. Prologue = batch 0's three
projections (dense PE while inputs land). Then 8 "duos" of attention pairs mixing
BOTH batches -- (b0p0,b0p1), (b0p2,b1p0), ..., (b1p6,b1p7) -- each duo's 8 stage
slots followed by one ~0.9us filler thunk: batch 1's projections fill the early
duos, batch 0's output projection fills the late duos, so the PE never idles long
enough for the HAM clock gate to re-throttle (attention alone is ScalarE-exp-bound
at ~8.6us/pair vs ~5.1us of PE work). imt/aug are streamed per-pair through small
rotating pools (2-duo lookahead) so both batches' working sets fit in SBUF.
ScalarE runs ONLY exp activations (plus prologue-phase evacuations and tail
stores); bulk loads ride Sync, per-pair loads ride GpSimd.
"""

import os
import sys

import numpy as np


def _ensure_concourse():
    try:
        import concourse.bass  # noqa: F401
        return
    except Exception:
        pass
    for p in ("/opt/trn_rl_repo", "/root/.axon_site/_ro/trn_rl_repo"):
        if os.path.isdir(p) and p not in sys.path:
            sys.path.insert(0, p)
            try:
                import concourse.bass  # noqa: F401
                return
            except Exception:
                sys.path.remove(p)
    raise ImportError("cannot import concourse (bass)")


B, L, HS = 16, 512, 1024
H, D = 16, 64
NCORES = 8
BPC = B // NCORES  # batches per core
SCALE = 1.0 / 8.0  # 1/sqrt(D)
AUGW = 65  # per-head augmented width (D + ones column)

_CACHE = {}


def _build_nc():
    _ensure_concourse()
    import concourse.bass as bass  # noqa: F401
    import concourse.mybir as mybir
    import concourse.tile as tile
    from concourse import bacc
    from contextlib import ExitStack

    bf = mybir.dt.bfloat16
    f32 = mybir.dt.float32
    Exp = mybir.ActivationFunctionType.Exp

    nc = bacc.Bacc()

    # all inputs host-preswizzled to [128, free] per-partition-contiguous
    # layouts so every load is one cheap 2D DMA
    qt = nc.declare_dram_parameter("qt", [BPC, 128, 8 * L], bf, isOutput=False)
    kt = nc.declare_dram_parameter("kt", [BPC, 128, 8 * L], bf, isOutput=False)
    vt = nc.declare_dram_parameter("vt", [BPC, 128, 8 * L], bf, isOutput=False)
    imt = nc.declare_dram_parameter("imt", [BPC, 128, 8 * L], bf, isOutput=False)
    aug = nc.declare_dram_parameter(
        "aug", [BPC, 128, 4 * H * AUGW], bf, isOutput=False
    )
    kp1 = nc.declare_dram_parameter("kp1", [BPC, 128, 4 * L], bf, isOutput=False)
    kp2 = nc.declare_dram_parameter("kp2", [BPC, 128, 4 * L], bf, isOutput=False)
    wq = nc.declare_dram_parameter("wq", [128, 8 * HS], bf, isOutput=False)
    wk = nc.declare_dram_parameter("wk", [128, 8 * HS], bf, isOutput=False)
    wv = nc.declare_dram_parameter("wv", [128, 8 * HS], bf, isOutput=False)
    wm = nc.declare_dram_parameter("wm", [128, 8 * HS], bf, isOutput=False)
    idt = nc.declare_dram_parameter("idt", [128, 128], bf, isOutput=False)
    out = nc.declare_dram_parameter("out", [BPC, L, HS], bf, isOutput=True)

    with ExitStack() as ctx:
        tc = ctx.enter_context(tile.TileContext(nc))
        consts = ctx.enter_context(tc.tile_pool(name="consts", bufs=1))
        inp = ctx.enter_context(tc.tile_pool(name="inp", bufs=1))
        rot = ctx.enter_context(tc.tile_pool(name="rot", bufs=6))
        proj = ctx.enter_context(tc.tile_pool(name="proj", bufs=2))
        ework = ctx.enter_context(tc.tile_pool(name="ework", bufs=2))
        small = ctx.enter_context(tc.tile_pool(name="small", bufs=3))
        evac = ctx.enter_context(tc.tile_pool(name="evac", bufs=2))
        psA = ctx.enter_context(tc.tile_pool(name="psA", bufs=3, space="PSUM"))
        psT = ctx.enter_context(tc.tile_pool(name="psT", bufs=2, space="PSUM"))

        w_sb = {}

        def load_weight_half(name, wext, half, tag=None):
            tag = tag or name
            if name not in w_sb:
                t = consts.tile([128, 8, HS], bf, tag=tag, name=name + "_sb")
                w_sb[name] = t
            t = w_sb[name]
            nc.sync.dma_start(
                out=t[:, half * 4 : (half + 1) * 4, :],
                in_=wext[:, half * 4 * HS : (half + 1) * 4 * HS],
            )

        # ---- bulk per-batch input tiles (qt/kt/vt shared between batches:
        # batch 1's load WAR-waits on batch 0's last projection reader) ----
        xin = {0: {}, 1: {}}

        def load_x_half(t, ext, b, half):
            # activation bulk loads ride the ScalarE HWDGE queue (exp-idle
            # during the prologue), in parallel with the weight stream on Sync
            nc.scalar.dma_start(
                out=t[:, half * 4 : (half + 1) * 4, :],
                in_=ext[b][:, half * 4 * L : (half + 1) * 4 * L],
            )

        def load_bulk(b, which):
            ext = {"qt": qt, "kt": kt, "vt": vt}[which]
            t = inp.tile([128, 8, L], bf, tag=which, name=which + "_sb")
            for half in range(2):
                load_x_half(t, ext, b, half)
            xin[b][which] = t

        def load_vt_jt(b):
            # jt-sliced vt load: 4 DMAs of [128, 8, 128] so the first vaug
            # group's slice lands after ~0.25MB instead of 1MB
            t = inp.tile([128, 8, L], bf, tag="vt", name="vt_sb")
            ext = vt[b].rearrange("p (c x) -> p c x", x=L)
            for jt in range(4):
                nc.scalar.dma_start(
                    out=t[:, :, jt * 128 : (jt + 1) * 128],
                    in_=ext[:, :, jt * 128 : (jt + 1) * 128],
                )
            xin[b]["vt"] = t

        def load_kp(b, which):
            ext = {"kp1": kp1, "kp2": kp2}[which]
            t = inp.tile([128, 4, L], bf, tag=f"{which}_{b}", name=which + "_sb")
            nc.gpsimd.dma_start(out=t, in_=ext[b])
            xin[b][which] = t

        # ---- per-pair rotating imt/aug tiles (2-duo lookahead) ----
        pair_imt = {}
        pair_aug = {}

        def load_pair(b, hp):
            ti = rot.tile([128, L], bf, tag="imt", name="imt_p")
            nc.gpsimd.dma_start(
                out=ti, in_=imt[b][:, hp * L : (hp + 1) * L]
            )
            pair_imt[(b, hp)] = ti
            ta = rot.tile([128, 4, 2 * AUGW], bf, tag="aug", name="aug_p")
            nc.gpsimd.dma_start(
                out=ta,
                in_=aug[b].rearrange("p (jt x) -> p jt x", x=H * AUGW)[
                    :, :, 2 * hp * AUGW : (2 * hp + 2) * AUGW
                ],
            )
            pair_aug[(b, hp)] = ta

        # ---- startup: wq/qt in fine-grained interleaved chunks so the first
        # projection matmuls can start after ~400KB instead of 3MB ----
        ident = consts.tile([128, 128], bf, tag="ident")
        xin[0]["qt"] = inp.tile([128, 8, L], bf, tag="qt", name="qt_sb")
        wq_t = consts.tile([128, 8, HS], bf, tag="wq", name="wq_sb")
        w_sb["wq"] = wq_t
        for c in range(4):
            nc.sync.dma_start(
                out=wq_t[:, 2 * c : 2 * c + 2, :],
                in_=wq[:, 2 * c * HS : (2 * c + 2) * HS],
            )
            nc.scalar.dma_start(
                out=xin[0]["qt"][:, 2 * c : 2 * c + 2, :],
                in_=qt[0][:, 2 * c * L : (2 * c + 2) * L],
            )
        nc.gpsimd.dma_start(out=ident, in_=idt[:, :])

        # ---- per-batch working tiles ----
        st = {}
        for b in range(BPC):
            st[b] = {
                "qh": proj.tile([128, 8, L], bf, tag="qh", name="qh_sb"),
                "kh": proj.tile([128, 8, L], bf, tag="kh", name="kh_sb"),
                "vaug": proj.tile(
                    [128, 4, H * AUGW], bf, tag="vaug", name="vaug_sb"
                ),
                "att": proj.tile([128, 8, L], bf, tag="att", name="att_sb"),
            }

        # ---- projection groups (one PSUM group each; interleavable thunks) ----
        def proj_qk_part(b, wname, dstname, ot, part, state):
            """half of a projection PSUM group (~0.9us of PE work)."""
            wt = w_sb[wname]
            xsb = xin[b]["qt" if wname == "wq" else "kt"]
            dst = st[b][dstname]
            if part == 0:
                state["ps"] = psA.tile([128, 512], f32, tag="psA", name="ps_pj")
            ps = state["ps"]
            for kc in range(4 * part, 4 * part + 4):
                nc.tensor.matmul(
                    ps,
                    wt[:, kc, ot * 128 : (ot + 1) * 128],
                    xsb[:, kc, :],
                    start=(kc == 0),
                    stop=(kc == 7),
                )
            if part == 1:
                if b == 0:
                    # b0's evacs ride ScalarE (exp-idle during the prologue);
                    # b1's (issued while ScalarE is dense with exp) ride DVE
                    nc.scalar.copy(out=dst[:, ot, :], in_=ps)
                else:
                    nc.vector.tensor_copy(out=dst[:, ot, :], in_=ps)

        def proj_qk_group(b, wname, dstname, ot):
            state = {}
            proj_qk_part(b, wname, dstname, ot, 0, state)
            proj_qk_part(b, wname, dstname, ot, 1, state)

        def vaug_part(b, jt, oh, part, state):
            """half of a v-projection group into the augmented layout."""
            vaug_sb = st[b]["vaug"]
            vt_sb = xin[b]["vt"]
            if part == 0 and oh == 0:
                nc.vector.memset(
                    vaug_sb[:, jt, :].rearrange("p (h x) -> p h x", x=AUGW)[
                        :, :, 64
                    ],
                    1.0,
                )
            if part == 0:
                state["ps"] = psA.tile([128, 512], f32, tag="psA", name="ps_pj")
            ps = state["ps"]
            for kc in range(4 * part, 4 * part + 4):
                nc.tensor.matmul(
                    ps,
                    vt_sb[:, kc, jt * 128 : (jt + 1) * 128],
                    w_sb["wv"][:, kc, oh * 512 : (oh + 1) * 512],
                    start=(kc == 0),
                    stop=(kc == 7),
                )
            if part == 1:
                dst_ap = vaug_sb[
                    :, jt, oh * 8 * AUGW : (oh + 1) * 8 * AUGW
                ].rearrange("p (h x) -> p h x", x=AUGW)[:, :, 0:64]
                nc.vector.tensor_copy(
                    out=dst_ap, in_=ps.rearrange("p (h x) -> p h x", x=64)
                )

        def vaug_group(b, jt, oh):
            state = {}
            vaug_part(b, jt, oh, 0, state)
            vaug_part(b, jt, oh, 1, state)

        def outproj_part(b, it, oh, part, state):
            """half of an output-projection PSUM group (~0.9us of PE work)."""
            att_sb = st[b]["att"]
            if part == 0:
                state["ps"] = psA.tile([128, 512], f32, tag="psA", name="ps_pj")
            ps = state["ps"]
            for kc in range(4 * part, 4 * part + 4):
                nc.tensor.matmul(
                    ps,
                    att_sb[:, kc, it * 128 : (it + 1) * 128],
                    w_sb["wm"][:, kc, oh * 512 : (oh + 1) * 512],
                    start=(kc == 0),
                    stop=(kc == 7),
                )
            if part == 1:
                ob = evac.tile([128, 512], bf, tag="ob")
                nc.vector.tensor_copy(out=ob, in_=ps)
                # batch 1's stores run in the kernel tail where ScalarE is
                # idle; batch 0's ride Sync during the attention region
                eng = nc.scalar if b == 1 else nc.sync
                eng.dma_start(
                    out=out[
                        b, it * 128 : (it + 1) * 128, oh * 512 : (oh + 1) * 512
                    ],
                    in_=ob,
                )

        def outproj_group(b, it, oh):
            state = {}
            outproj_part(b, it, oh, 0, state)
            outproj_part(b, it, oh, 1, state)

        # ---- attention stages ----
        def score_stage(b, hp, lhs_fn, rhs_fn, etile):
            """s^T [j,i] for both heads of pair hp + exp into etile.

            The two heads' matmuls use lhsT base partitions 0 / 64, so they
            run concurrently on the two row-halves of the PE array (outputs
            land in different PSUM banks)."""
            for jt in range(4):
                ps = psA.tile([128, 1024], f32, tag="psA", name="ps_sc")
                for g in range(2):
                    nc.tensor.matmul(
                        ps[:, g * 512 : (g + 1) * 512],
                        lhs_fn(g, jt),
                        rhs_fn(g),
                        start=True,
                        stop=True,
                    )
                nc.scalar.activation(
                    out=etile[:, jt],
                    in_=ps.rearrange("p (g x) -> p g x", x=512),
                    func=Exp,
                    scale=SCALE,
                )

        def mask_stage(b, hp, kp_sb, etile):
            # in-place mask multiply, one fused op per jt covering both heads
            # (mask row broadcast across the head dim via a stride-0 AP).
            # All on DVE: a GpSimd-offloaded op costs ~2.2us on the
            # exp->mask->pv critical chain and stalls the in-order PE queue.
            for jt in range(4):
                kpb = kp_sb[:, jt, :].unsqueeze(1).broadcast_to([128, 2, L])
                nc.vector.tensor_mul(etile[:, jt], etile[:, jt], kpb)

        def pv_stage(b, hp, emtile, rhs_fn):
            """pv natural [i, 4*65] per head -> normalized dl pair [128,4,128].

            One fused broadcast-multiply per head turns the raw PSUM pv tile
            into the normalized bf16 dl tile (recip row-sums broadcast along
            d via a stride-0 AP)."""
            dl = small.tile([128, 4, 128], bf, tag="dl")
            for g in range(2):
                pspv = psT.tile([128, 4, AUGW], f32, tag="tail")
                for it in range(4):
                    for jt in range(4):
                        nc.tensor.matmul(
                            pspv[:, it, :],
                            emtile[:, jt, g, it * 128 : (it + 1) * 128],
                            rhs_fn(g, jt),
                            start=(jt == 0),
                            stop=(jt == 3),
                        )
                r1 = small.tile([128, 4], f32, tag="r1")
                nc.vector.reciprocal(r1, pspv[:, :, 64])
                nc.vector.tensor_mul(
                    dl[:, :, g * 64 : (g + 1) * 64],
                    pspv[:, :, 0:64],
                    r1.unsqueeze(-1).broadcast_to([128, 4, 64]),
                )
            return dl

        def s1_stage(b, hp):
            e1 = ework.tile([128, 4, 2, L], bf, tag="e")
            imt_t = pair_imt[(b, hp)]
            qh_sb = st[b]["qh"]
            score_stage(
                b,
                hp,
                lambda g, jt: imt_t[g * 64 : g * 64 + 64, jt * 128 : (jt + 1) * 128],
                lambda g: qh_sb[g * 64 : g * 64 + 64, hp, :],
                e1,
            )
            mask_stage(b, hp, xin[b]["kp1"], e1)
            return e1

        def mod_stage(b, hp, e1):
            """round-1 tail: pv + normalize + PE-transpose + add qh -> qn_pair.

            This transpose is on the s2 latency chain, so it stays on TensorE
            instead of the ~1.2us-per-op DMA xbar path."""
            aug_t = pair_aug[(b, hp)]
            dl = pv_stage(
                b, hp, e1,
                lambda g, jt: aug_t[:, jt, g * AUGW : (g + 1) * AUGW],
            )
            pst = psT.tile([128, 512], bf, tag="tail", name="pst")
            for it in range(4):
                nc.tensor.transpose(
                    pst[:, it * 128 : (it + 1) * 128], dl[:, it, :], ident
                )
            qn_pair = small.tile([128, 512], bf, tag="qnp")
            nc.vector.tensor_add(qn_pair, pst, st[b]["qh"][:, hp, :])
            return qn_pair

        def s2_stage(b, hp, qn_pair):
            e2 = ework.tile([128, 4, 2, L], bf, tag="e")
            kh_sb = st[b]["kh"]
            score_stage(
                b,
                hp,
                lambda g, jt: kh_sb[g * 64 : g * 64 + 64, hp, jt * 128 : (jt + 1) * 128],
                lambda g: qn_pair[g * 64 : g * 64 + 64, :],
                e2,
            )
            mask_stage(b, hp, xin[b]["kp2"], e2)
            return e2

        def av_stage(b, hp, e2):
            """round-2 tail: av + normalize + DMA-transpose -> att[:, hp, :]."""
            vaug_sb = st[b]["vaug"]
            dl = pv_stage(
                b, hp, e2,
                lambda g, jt: vaug_sb[:, jt, (2 * hp + g) * AUGW : (2 * hp + g + 1) * AUGW],
            )
            for it in range(4):
                nc.sync.dma_start_transpose(
                    out=st[b]["att"][:, hp, it * 128 : (it + 1) * 128],
                    in_=dl[:, it, :],
                )

        # ---- stage-stream runner ----
        def do_s1(p):
            p["e1"] = s1_stage(p["b"], p["hp"])

        def do_mod(p):
            p["qn"] = mod_stage(p["b"], p["hp"], p["e1"])

        def do_s2(p):
            p["e2"] = s2_stage(p["b"], p["hp"], p["qn"])

        def do_av(p):
            av_stage(p["b"], p["hp"], p["e2"])

        def run_duos(duos, fillers, preduo_hooks):
            for di, (pa, pb) in enumerate(duos):
                for hook in preduo_hooks.get(di, ()):
                    hook()
                for fn, p in (
                    (do_s1, pa), (do_s1, pb),
                    (do_mod, pa), (do_mod, pb),
                    (do_s2, pa), (do_s2, pb),
                    (do_av, pa), (do_av, pb),
                ):
                    if p is not None:
                        fn(p)
                    if fillers:
                        t = fillers.pop(0)
                        if t is not None:
                            t()
            while fillers:
                t = fillers.pop(0)
                if t is not None:
                    t()

        # ---- schedule ----
        # prologue: batch 0's three projections, dense PE while inputs land.
        # qh first (so qt(b1)'s WAR clears early), then kh, then vaug.
        # Weight stream on Sync; activation bulk loads on DVE's queue
        # (parallel DMA queues halve the prologue's DMA critical path);
        # per-pair imt/aug + keep-masks on GpSimd's queue.
        load_kp(0, "kp1")
        load_kp(0, "kp2")
        # imt/aug for duo0+duo1's pairs (hooks cover duo2 onward)
        for p_ in ((0, 0), (0, 1), (0, 2), (1, 0)):
            load_pair(*p_)
        kt0 = inp.tile([128, 8, L], bf, tag="kt", name="kt_sb")
        xin[0]["kt"] = kt0
        vt0 = inp.tile([128, 8, L], bf, tag="vt", name="vt_sb")
        xin[0]["vt"] = vt0
        for ot in range(8):
            if ot < 2:
                load_weight_half("wk", wk, ot)
                load_x_half(kt0, kt, 0, ot)
            elif ot < 4:
                load_weight_half("wv", wv, ot - 2)
                load_x_half(vt0, vt, 0, ot - 2)
            elif ot < 6:
                load_weight_half("wm", wm, ot - 4)
            proj_qk_group(0, "wq", "qh", ot)
        load_bulk(1, "qt")  # WAR on b0's qh groups just cleared
        load_kp(1, "kp1")
        for ot in range(8):
            proj_qk_group(0, "wk", "kh", ot)
        load_bulk(1, "kt")
        load_kp(1, "kp2")
        for jt in range(4):
            for oh in range(2):
                vaug_group(0, jt, oh)
        load_vt_jt(1)

        P = {}
        for b in range(2):
            for hp in range(8):
                P[(b, hp)] = {"b": b, "hp": hp}

        duos = [
            (P[(0, 0)], P[(0, 1)]),
            (P[(0, 2)], P[(1, 0)]),
            (P[(0, 3)], P[(1, 1)]),
            (P[(0, 4)], P[(1, 2)]),
            (P[(0, 5)], P[(1, 3)]),
            (P[(0, 6)], P[(0, 7)]),
            (P[(1, 4)], P[(1, 5)]),
            (P[(1, 6)], P[(1, 7)]),
        ]

        def halves(fn, *args):
            state = {}
            return [
                (lambda part=part, state=state: fn(*args, part, state))
                for part in range(2)
            ]

        fillers = []
        # duo0: b1's first qh/kh groups + first half of vaug-oh0
        fillers += halves(proj_qk_part, 1, "wq", "qh", 0)
        fillers += halves(proj_qk_part, 1, "wk", "kh", 0)
        fillers += halves(vaug_part, 1, 0, 0)
        fillers += halves(vaug_part, 1, 1, 0)
        # duo1: rest of vaug-oh0 (b1p0's av needs all jt), then ot1
        fillers += halves(vaug_part, 1, 2, 0)
        fillers += halves(vaug_part, 1, 3, 0)
        fillers += halves(proj_qk_part, 1, "wq", "qh", 1)
        fillers += halves(proj_qk_part, 1, "wk", "kh", 1)
        # duo2-4: qh/kh ot2..7
        for ot in range(2, 8):
            fillers += halves(proj_qk_part, 1, "wq", "qh", ot)
            fillers += halves(proj_qk_part, 1, "wk", "kh", ot)
        # duo5: vaug-oh1 (b1p4's av in duo6 needs it)
        for jt in range(4):
            fillers += halves(vaug_part, 1, jt, 1)
        # duo6-7: b0's output projection (b0p7's av ends in duo5)
        for it in range(4):
            for oh in range(2):
                fillers += halves(outproj_part, 0, it, oh)

        hooks = {}
        for di in range(len(duos)):
            hs = []
            if di + 2 < len(duos):
                for p in duos[di + 2]:
                    if p is not None:
                        hs.append(
                            lambda b=p["b"], hp=p["hp"]: load_pair(b, hp)
                        )
            hooks[di] = hs

        run_duos(duos, fillers, hooks)

        # tail: batch 1's output projection (ScalarE idle -> stores on scalar)
        for it in range(4):
            for oh in range(2):
                outproj_group(1, it, oh)

    nc.compile()
    return nc


def _get_nc():
    if "nc" not in _CACHE:
        _CACHE["nc"] = _build_nc()
    return _CACHE["nc"]


def _prep_inputs(v, k, q, img_abs, Wv, Wk, Wq, Wm, abs_mask, mask):
    import ml_dtypes

    bf16 = ml_dtypes.bfloat16
    f32 = np.float32

    def swz(x, nt):  # [B, nt*128, F] -> [B, 128, nt*F] partition-contiguous
        b, r, f = x.shape
        return np.ascontiguousarray(
            x.reshape(b, nt, 128, f).transpose(0, 2, 1, 3).reshape(b, 128, nt * f)
        )

    def t_bf(x):  # [B, L, HS] -> [B, 128, 8*L] bf16 swizzled
        xt = np.swapaxes(np.asarray(x, f32), 1, 2)
        return swz(xt, 8).astype(bf16)

    qt = t_bf(q)
    ktr = t_bf(k)
    vtr = t_bf(v)
    imt = t_bf(img_abs)

    img = np.asarray(img_abs, f32)
    augf = np.empty((B, L, H * AUGW), f32)
    augf.reshape(B, L, H, AUGW)[..., :64] = img.reshape(B, L, H, 64)
    augf.reshape(B, L, H, AUGW)[..., 64] = 1.0
    augv = swz(augf, 4).astype(bf16)

    def keepT(m):  # [B, 1, L, L] bool -> (1-m)^T swizzled bf16
        kf = 1.0 - np.asarray(m, f32)[:, 0]
        return swz(np.swapaxes(kf, 1, 2), 4).astype(bf16)

    kp1 = keepT(abs_mask)
    kp2 = keepT(mask)

    def wT(w):
        wt = np.asarray(w, f32).T  # [i, o]
        return swz(wt[None], 8)[0].astype(bf16)

    wqs, wks, wvs, wms = wT(Wq), wT(Wk), wT(Wv), wT(Wm)
    ident = np.eye(128, dtype=bf16)

    in_maps = []
    for c in range(NCORES):
        s = slice(c * BPC, (c + 1) * BPC)
        in_maps.append(
            {
                "qt": qt[s],
                "kt": ktr[s],
                "vt": vtr[s],
                "imt": imt[s],
                "aug": augv[s],
                "kp1": kp1[s],
                "kp2": kp2[s],
                "wq": wqs,
                "wk": wks,
                "wv": wvs,
                "wm": wms,
                "idt": ident,
            }
        )
    return in_maps


def kernel(v, k, q, img_abs, Wv, Wk, Wq, Wm, abs_mask, mask, _trace=False):
    _ensure_concourse()
    from concourse.bass_utils import run_bass_kernel_spmd

    in_maps = _prep_inputs(v, k, q, img_abs, Wv, Wk, Wq, Wm, abs_mask, mask)
    nc = _get_nc()
    res = run_bass_kernel_spmd(nc, in_maps, core_ids=list(range(NCORES)), trace=_trace)
    outp = np.concatenate([res.results[i]["out"] for i in range(NCORES)], axis=0)
    outp = np.asarray(outp, np.float32)  # device stores bf16; upcast on host
    if _trace:
        _CACHE["last_result"] = res
    return outp


# revision 25
# speedup vs baseline: 1.1317x; 1.0386x over previous
"""Trainium2 Bass kernel for nn_ABS_MHAtt (masked two-round multi-head attention).

Strategy: pure data-parallel over batch (B=16 -> 2 batches per NeuronCore, 8 cores,
no collectives). Host-side preprocessing (inside kernel()) pre-transposes
activations/weights into the [contraction, free] layouts the TensorEngine wants and
pre-converts everything to bf16, so the device kernel does zero layout conversion.

Per-core device kernel (per batch):
  - qhT/khT projections in transposed form [o, i]; v projected in natural form [j, o]
    directly into an "augmented" layout with a ones column per head (the ones column
    makes the PV/AV matmul also produce the softmax row-sum).
  - Per head: scores computed transposed [j, i] (contraction over d=64, head pairs
    row-tiled onto the two PE array halves), exp on ScalarE, masking by multiplying
    with (1-mask)^T (split across VectorE and GpSimdE), PV/AV with E as the
    stationary operand, one fused broadcast-multiply normalize per head, and DMA
    xbar transposes (not TensorE) to repack [i, d] tiles back to [d, i].

v3 scheduling: single interleaved stage stream. Prologue = batch 0's three
projections (dense PE while inputs land). Then 8 "duos" of attention pairs mixing
BOTH batches -- (b0p0,b0p1), (b0p2,b1p0), ..., (b1p6,b1p7) -- each duo's 8 stage
slots followed by one ~0.9us filler thunk: batch 1's projections fill the early
duos, batch 0's output projection fills the late duos, so the PE never idles long
enough for the HAM clock gate to re-throttle (attention alone is ScalarE-exp-bound
at ~8.6us/pair vs ~5.1us of PE work). imt/aug are streamed per-pair through small
rotating pools (2-duo lookahead) so both batches' working sets fit in SBUF.
ScalarE runs ONLY exp activations (plus prologue-phase evacuations and tail
stores); bulk loads ride Sync, per-pair loads ride GpSimd.
"""

import os
import sys

import numpy as np


def _ensure_concourse():
    try:
        import concourse.bass  # noqa: F401
        return
    except Exception:
        pass
    for p in ("/opt/trn_rl_repo", "/root/.axon_site/_ro/trn_rl_repo"):
        if os.path.isdir(p) and p not in sys.path:
            sys.path.insert(0, p)
            try:
                import concourse.bass  # noqa: F401
                return
            except Exception:
                sys.path.remove(p)
    raise ImportError("cannot import concourse (bass)")


B, L, HS = 16, 512, 1024
H, D = 16, 64
NCORES = 8
BPC = B // NCORES  # batches per core
SCALE = 1.0 / 8.0  # 1/sqrt(D)
AUGW = 65  # per-head augmented width (D + ones column)

_CACHE = {}


def _build_nc():
    _ensure_concourse()
    import concourse.bass as bass  # noqa: F401
    import concourse.mybir as mybir
    import concourse.tile as tile
    from concourse import bacc
    from contextlib import ExitStack

    bf = mybir.dt.bfloat16
    f32 = mybir.dt.float32
    Exp = mybir.ActivationFunctionType.Exp

    nc = bacc.Bacc()

    # all inputs host-preswizzled to [128, free] per-partition-contiguous
    # layouts so every load is one cheap 2D DMA
    qt = nc.declare_dram_parameter("qt", [BPC, 128, 8 * L], bf, isOutput=False)
    kt = nc.declare_dram_parameter("kt", [BPC, 128, 8 * L], bf, isOutput=False)
    vt = nc.declare_dram_parameter("vt", [BPC, 128, 8 * L], bf, isOutput=False)
    imt = nc.declare_dram_parameter("imt", [BPC, 128, 8 * L], bf, isOutput=False)
    aug = nc.declare_dram_parameter(
        "aug", [BPC, 128, 4 * H * AUGW], bf, isOutput=False
    )
    kp1 = nc.declare_dram_parameter("kp1", [BPC, 128, 4 * L], bf, isOutput=False)
    kp2 = nc.declare_dram_parameter("kp2", [BPC, 128, 4 * L], bf, isOutput=False)
    wq = nc.declare_dram_parameter("wq", [128, 8 * HS], bf, isOutput=False)
    wk = nc.declare_dram_parameter("wk", [128, 8 * HS], bf, isOutput=False)
    wv = nc.declare_dram_parameter("wv", [128, 8 * HS], bf, isOutput=False)
    wm = nc.declare_dram_parameter("wm", [128, 8 * HS], bf, isOutput=False)
    idt = nc.declare_dram_parameter("idt", [128, 128], bf, isOutput=False)
    out = nc.declare_dram_parameter("out", [BPC, L, HS], bf, isOutput=True)

    with ExitStack() as ctx:
        tc = ctx.enter_context(tile.TileContext(nc))
        consts = ctx.enter_context(tc.tile_pool(name="consts", bufs=1))
        inp = ctx.enter_context(tc.tile_pool(name="inp", bufs=1))
        rot = ctx.enter_context(tc.tile_pool(name="rot", bufs=6))
        proj = ctx.enter_context(tc.tile_pool(name="proj", bufs=2))
        ework = ctx.enter_context(tc.tile_pool(name="ework", bufs=2))
        small = ctx.enter_context(tc.tile_pool(name="small", bufs=3))
        evac = ctx.enter_context(tc.tile_pool(name="evac", bufs=2))
        psA = ctx.enter_context(tc.tile_pool(name="psA", bufs=3, space="PSUM"))
        psT = ctx.enter_context(tc.tile_pool(name="psT", bufs=2, space="PSUM"))

        w_sb = {}

        def load_weight_half(name, wext, half, tag=None):
            tag = tag or name
            if name not in w_sb:
                t = consts.tile([128, 8, HS], bf, tag=tag, name=name + "_sb")
                w_sb[name] = t
            t = w_sb[name]
            nc.sync.dma_start(
                out=t[:, half * 4 : (half + 1) * 4, :],
                in_=wext[:, half * 4 * HS : (half + 1) * 4 * HS],
            )

        # ---- bulk per-batch input tiles (qt/kt/vt shared between batches:
        # batch 1's load WAR-waits on batch 0's last projection reader) ----
        xin = {0: {}, 1: {}}

        def load_x_half(t, ext, b, half):
            # batch 0's bulk loads ride the ScalarE HWDGE queue (exp-idle
            # during the prologue), parallel to the weight stream on Sync;
            # batch 1's ride GpSimd so they don't crowd b0's critical path
            eng = nc.scalar if b == 0 else nc.gpsimd
            eng.dma_start(
                out=t[:, half * 4 : (half + 1) * 4, :],
                in_=ext[b][:, half * 4 * L : (half + 1) * 4 * L],
            )

        def load_bulk(b, which):
            ext = {"qt": qt, "kt": kt, "vt": vt}[which]
            t = inp.tile([128, 8, L], bf, tag=which, name=which + "_sb")
            for half in range(2):
                load_x_half(t, ext, b, half)
            xin[b][which] = t

        def load_vt_jt(b):
            # jt-sliced vt load: 4 DMAs of [128, 8, 128] so the first vaug
            # group's slice lands after ~0.25MB instead of 1MB
            t = inp.tile([128, 8, L], bf, tag="vt", name="vt_sb")
            ext = vt[b].rearrange("p (c x) -> p c x", x=L)
            eng = nc.scalar if b == 0 else nc.gpsimd
            for jt in range(4):
                eng.dma_start(
                    out=t[:, :, jt * 128 : (jt + 1) * 128],
                    in_=ext[:, :, jt * 128 : (jt + 1) * 128],
                )
            xin[b]["vt"] = t

        def load_kp(b, which):
            ext = {"kp1": kp1, "kp2": kp2}[which]
            t = inp.tile([128, 4, L], bf, tag=f"{which}_{b}", name=which + "_sb")
            nc.gpsimd.dma_start(out=t, in_=ext[b])
            xin[b][which] = t

        # ---- per-pair rotating imt/aug tiles (2-duo lookahead) ----
        pair_imt = {}
        pair_aug = {}

        def load_pair(b, hp):
            ti = rot.tile([128, L], bf, tag="imt", name="imt_p")
            nc.gpsimd.dma_start(
                out=ti, in_=imt[b][:, hp * L : (hp + 1) * L]
            )
            pair_imt[(b, hp)] = ti
            ta = rot.tile([128, 4, 2 * AUGW], bf, tag="aug", name="aug_p")
            nc.gpsimd.dma_start(
                out=ta,
                in_=aug[b].rearrange("p (jt x) -> p jt x", x=H * AUGW)[
                    :, :, 2 * hp * AUGW : (2 * hp + 2) * AUGW
                ],
            )
            pair_aug[(b, hp)] = ta

        # ---- startup: wq/qt in fine-grained interleaved chunks so the first
        # projection matmuls can start after ~400KB instead of 3MB ----
        ident = consts.tile([128, 128], bf, tag="ident")
        xin[0]["qt"] = inp.tile([128, 8, L], bf, tag="qt", name="qt_sb")
        wq_t = consts.tile([128, 8, HS], bf, tag="wq", name="wq_sb")
        w_sb["wq"] = wq_t
        for c in range(4):
            nc.sync.dma_start(
                out=wq_t[:, 2 * c : 2 * c + 2, :],
                in_=wq[:, 2 * c * HS : (2 * c + 2) * HS],
            )
            nc.scalar.dma_start(
                out=xin[0]["qt"][:, 2 * c : 2 * c + 2, :],
                in_=qt[0][:, 2 * c * L : (2 * c + 2) * L],
            )
        nc.gpsimd.dma_start(out=ident, in_=idt[:, :])

        # ---- per-batch working tiles ----
        st = {}
        for b in range(BPC):
            st[b] = {
                "qh": proj.tile([128, 8, L], bf, tag="qh", name="qh_sb"),
                "kh": proj.tile([128, 8, L], bf, tag="kh", name="kh_sb"),
                "vaug": proj.tile(
                    [128, 4, H * AUGW], bf, tag="vaug", name="vaug_sb"
                ),
                "att": proj.tile([128, 8, L], bf, tag="att", name="att_sb"),
            }

        # ---- projection groups (one PSUM group each; interleavable thunks) ----
        def proj_qk_part(b, wname, dstname, ot, part, state):
            """half of a projection PSUM group (~0.9us of PE work)."""
            wt = w_sb[wname]
            xsb = xin[b]["qt" if wname == "wq" else "kt"]
            dst = st[b][dstname]
            if part == 0:
                state["ps"] = psA.tile([128, 512], f32, tag="psA", name="ps_pj")
            ps = state["ps"]
            for kc in range(4 * part, 4 * part + 4):
                nc.tensor.matmul(
                    ps,
                    wt[:, kc, ot * 128 : (ot + 1) * 128],
                    xsb[:, kc, :],
                    start=(kc == 0),
                    stop=(kc == 7),
                )
            if part == 1:
                if b == 0:
                    # b0's evacs ride ScalarE (exp-idle during the prologue);
                    # b1's ride DVE (GpSimd has no PSUM port)
                    nc.scalar.copy(out=dst[:, ot, :], in_=ps)
                else:
                    nc.vector.tensor_copy(out=dst[:, ot, :], in_=ps)

        def proj_qk_group(b, wname, dstname, ot):
            state = {}
            proj_qk_part(b, wname, dstname, ot, 0, state)
            proj_qk_part(b, wname, dstname, ot, 1, state)

        def vaug_part(b, jt, oh, part, state):
            """half of a v-projection group into the augmented layout."""
            vaug_sb = st[b]["vaug"]
            vt_sb = xin[b]["vt"]
            if part == 0 and oh == 0:
                nc.vector.memset(
                    vaug_sb[:, jt, :].rearrange("p (h x) -> p h x", x=AUGW)[
                        :, :, 64
                    ],
                    1.0,
                )
            if part == 0:
                state["ps"] = psA.tile([128, 512], f32, tag="psA", name="ps_pj")
            ps = state["ps"]
            for kc in range(4 * part, 4 * part + 4):
                nc.tensor.matmul(
                    ps,
                    vt_sb[:, kc, jt * 128 : (jt + 1) * 128],
                    w_sb["wv"][:, kc, oh * 512 : (oh + 1) * 512],
                    start=(kc == 0),
                    stop=(kc == 7),
                )
            if part == 1:
                dst_ap = vaug_sb[
                    :, jt, oh * 8 * AUGW : (oh + 1) * 8 * AUGW
                ].rearrange("p (h x) -> p h x", x=AUGW)[:, :, 0:64]
                nc.vector.tensor_copy(
                    out=dst_ap, in_=ps.rearrange("p (h x) -> p h x", x=64)
                )

        def vaug_group(b, jt, oh):
            state = {}
            vaug_part(b, jt, oh, 0, state)
            vaug_part(b, jt, oh, 1, state)

        def outproj_part(b, it, oh, part, state):
            """half of an output-projection PSUM group (~0.9us of PE work)."""
            att_sb = st[b]["att"]
            if part == 0:
                state["ps"] = psA.tile([128, 512], f32, tag="psA", name="ps_pj")
            ps = state["ps"]
            for kc in range(4 * part, 4 * part + 4):
                nc.tensor.matmul(
                    ps,
                    att_sb[:, kc, it * 128 : (it + 1) * 128],
                    w_sb["wm"][:, kc, oh * 512 : (oh + 1) * 512],
                    start=(kc == 0),
                    stop=(kc == 7),
                )
            if part == 1:
                ob = evac.tile([128, 512], bf, tag="ob")
                nc.vector.tensor_copy(out=ob, in_=ps)
                # batch 1's stores run in the kernel tail where ScalarE and
                # GpSimd are idle (alternate queues so the ~0.6us trigger
                # instructions don't serialize); batch 0's ride Sync
                if b == 1:
                    eng = nc.scalar if (it + oh) % 2 == 0 else nc.gpsimd
                else:
                    eng = nc.sync
                eng.dma_start(
                    out=out[
                        b, it * 128 : (it + 1) * 128, oh * 512 : (oh + 1) * 512
                    ],
                    in_=ob,
                )

        def outproj_group(b, it, oh):
            state = {}
            outproj_part(b, it, oh, 0, state)
            outproj_part(b, it, oh, 1, state)

        # ---- attention stages ----
        def score_stage(b, hp, lhs_fn, rhs_fn, etile):
            """s^T [j,i] for both heads of pair hp + exp into etile.

            The two heads' matmuls use lhsT base partitions 0 / 64, so they
            run concurrently on the two row-halves of the PE array (outputs
            land in different PSUM banks)."""
            for jt in range(4):
                ps = psA.tile([128, 1024], f32, tag="psA", name="ps_sc")
                for g in range(2):
                    nc.tensor.matmul(
                        ps[:, g * 512 : (g + 1) * 512],
                        lhs_fn(g, jt),
                        rhs_fn(g),
                        start=True,
                        stop=True,
                    )
                nc.scalar.activation(
                    out=etile[:, jt],
                    in_=ps.rearrange("p (g x) -> p g x", x=512),
                    func=Exp,
                    scale=SCALE,
                )

        def mask_stage(b, hp, kp_sb, etile):
            # in-place mask multiply, one fused op per jt covering both heads
            # (mask row broadcast across the head dim via a stride-0 AP).
            # All on DVE: a GpSimd-offloaded op costs ~2.2us on the
            # exp->mask->pv critical chain and stalls the in-order PE queue.
            for jt in range(4):
                kpb = kp_sb[:, jt, :].unsqueeze(1).broadcast_to([128, 2, L])
                nc.vector.tensor_mul(etile[:, jt], etile[:, jt], kpb)

        def pv_stage(b, hp, emtile, rhs_fn):
            """pv natural [i, 4*65] per head -> normalized dl pair [128,4,128].

            One fused broadcast-multiply per head turns the raw PSUM pv tile
            into the normalized bf16 dl tile (recip row-sums broadcast along
            d via a stride-0 AP)."""
            dl = small.tile([128, 4, 128], bf, tag="dl")
            for g in range(2):
                pspv = psT.tile([128, 4, AUGW], f32, tag="tail")
                for it in range(4):
                    for jt in range(4):
                        nc.tensor.matmul(
                            pspv[:, it, :],
                            emtile[:, jt, g, it * 128 : (it + 1) * 128],
                            rhs_fn(g, jt),
                            start=(jt == 0),
                            stop=(jt == 3),
                        )
                r1 = small.tile([128, 4], f32, tag="r1")
                nc.vector.reciprocal(r1, pspv[:, :, 64])
                nc.vector.tensor_mul(
                    dl[:, :, g * 64 : (g + 1) * 64],
                    pspv[:, :, 0:64],
                    r1.unsqueeze(-1).broadcast_to([128, 4, 64]),
                )
            return dl

        def s1_stage(b, hp):
            e1 = ework.tile([128, 4, 2, L], bf, tag="e")
            imt_t = pair_imt[(b, hp)]
            qh_sb = st[b]["qh"]
            score_stage(
                b,
                hp,
                lambda g, jt: imt_t[g * 64 : g * 64 + 64, jt * 128 : (jt + 1) * 128],
                lambda g: qh_sb[g * 64 : g * 64 + 64, hp, :],
                e1,
            )
            mask_stage(b, hp, xin[b]["kp1"], e1)
            return e1

        def mod_stage(b, hp, e1):
            """round-1 tail: pv + normalize + PE-transpose + add qh -> qn_pair.

            This transpose is on the s2 latency chain, so it stays on TensorE
            instead of the ~1.2us-per-op DMA xbar path."""
            aug_t = pair_aug[(b, hp)]
            dl = pv_stage(
                b, hp, e1,
                lambda g, jt: aug_t[:, jt, g * AUGW : (g + 1) * AUGW],
            )
            pst = psT.tile([128, 512], bf, tag="tail", name="pst")
            for it in range(4):
                nc.tensor.transpose(
                    pst[:, it * 128 : (it + 1) * 128], dl[:, it, :], ident
                )
            qn_pair = small.tile([128, 512], bf, tag="qnp")
            nc.vector.tensor_add(qn_pair, pst, st[b]["qh"][:, hp, :])
            return qn_pair

        def s2_stage(b, hp, qn_pair):
            e2 = ework.tile([128, 4, 2, L], bf, tag="e")
            kh_sb = st[b]["kh"]
            score_stage(
                b,
                hp,
                lambda g, jt: kh_sb[g * 64 : g * 64 + 64, hp, jt * 128 : (jt + 1) * 128],
                lambda g: qn_pair[g * 64 : g * 64 + 64, :],
                e2,
            )
            mask_stage(b, hp, xin[b]["kp2"], e2)
            return e2

        def av_stage(b, hp, e2):
            """round-2 tail: av + normalize + transpose -> att[:, hp, :].

            Normally via DMA xbar (off the engines); the LAST duo's pairs use
            TensorE transposes instead -- their repack gates the tail output
            projection, and the xbar path costs ~1.2us per tile plus sync-
            queue waits right when the tail needs att complete."""
            vaug_sb = st[b]["vaug"]
            dl = pv_stage(
                b, hp, e2,
                lambda g, jt: vaug_sb[:, jt, (2 * hp + g) * AUGW : (2 * hp + g + 1) * AUGW],
            )
            if b == 1 and hp >= 6:
                pst = psT.tile([128, 512], bf, tag="tail", name="pst")
                for it in range(4):
                    nc.tensor.transpose(
                        pst[:, it * 128 : (it + 1) * 128], dl[:, it, :], ident
                    )
                nc.vector.tensor_copy(out=st[b]["att"][:, hp, :], in_=pst)
            else:
                for it in range(4):
                    nc.sync.dma_start_transpose(
                        out=st[b]["att"][:, hp, it * 128 : (it + 1) * 128],
                        in_=dl[:, it, :],
                    )

        # ---- stage-stream runner ----
        def do_s1(p):
            p["e1"] = s1_stage(p["b"], p["hp"])

        def do_mod(p):
            p["qn"] = mod_stage(p["b"], p["hp"], p["e1"])

        def do_s2(p):
            p["e2"] = s2_stage(p["b"], p["hp"], p["qn"])

        def do_av(p):
            av_stage(p["b"], p["hp"], p["e2"])

        def run_duos(duos, fillers, preduo_hooks):
            for di, (pa, pb) in enumerate(duos):
                for hook in preduo_hooks.get(di, ()):
                    hook()
                for fn, p in (
                    (do_s1, pa), (do_s1, pb),
                    (do_mod, pa), (do_mod, pb),
                    (do_s2, pa), (do_s2, pb),
                    (do_av, pa), (do_av, pb),
                ):
                    if p is not None:
                        fn(p)
                    if fillers:
                        t = fillers.pop(0)
                        if t is not None:
                            t()
            while fillers:
                t = fillers.pop(0)
                if t is not None:
                    t()

        # ---- schedule ----
        # prologue: batch 0's three projections, dense PE while inputs land.
        # qh first (so qt(b1)'s WAR clears early), then kh, then vaug.
        # Weight stream on Sync; activation bulk loads on DVE's queue
        # (parallel DMA queues halve the prologue's DMA critical path);
        # per-pair imt/aug + keep-masks on GpSimd's queue.
        load_kp(0, "kp1")
        load_kp(0, "kp2")
        # imt/aug for duo0+duo1's pairs (hooks cover duo2 onward)
        for p_ in ((0, 0), (0, 1), (0, 2), (1, 0)):
            load_pair(*p_)
        kt0 = inp.tile([128, 8, L], bf, tag="kt", name="kt_sb")
        xin[0]["kt"] = kt0
        vt0 = inp.tile([128, 8, L], bf, tag="vt", name="vt_sb")
        xin[0]["vt"] = vt0
        for ot in range(8):
            if ot < 2:
                load_weight_half("wk", wk, ot)
                load_x_half(kt0, kt, 0, ot)
            elif ot < 4:
                load_weight_half("wv", wv, ot - 2)
                load_x_half(vt0, vt, 0, ot - 2)
            proj_qk_group(0, "wq", "qh", ot)
        load_bulk(1, "qt")  # WAR on b0's qh groups just cleared
        load_kp(1, "kp1")
        for ot in range(8):
            proj_qk_group(0, "wk", "kh", ot)
        load_bulk(1, "kt")
        load_kp(1, "kp2")
        for jt in range(4):
            for oh in range(2):
                vaug_group(0, jt, oh)
        load_vt_jt(1)

        P = {}
        for b in range(2):
            for hp in range(8):
                P[(b, hp)] = {"b": b, "hp": hp}

        duos = [
            (P[(0, 0)], P[(0, 1)]),
            (P[(0, 2)], P[(1, 0)]),
            (P[(0, 3)], P[(1, 1)]),
            (P[(0, 4)], P[(1, 2)]),
            (P[(0, 5)], P[(1, 3)]),
            (P[(0, 6)], P[(0, 7)]),
            (P[(1, 4)], P[(1, 5)]),
            (P[(1, 6)], P[(1, 7)]),
        ]

        def halves(fn, *args):
            state = {}
            return [
                (lambda part=part, state=state: fn(*args, part, state))
                for part in range(2)
            ]

        fillers = []
        # duo0: b1's first qh/kh groups + first half of vaug-oh0
        fillers += halves(proj_qk_part, 1, "wq", "qh", 0)
        fillers += halves(proj_qk_part, 1, "wk", "kh", 0)
        fillers += halves(vaug_part, 1, 0, 0)
        fillers += halves(vaug_part, 1, 1, 0)
        # duo1: rest of vaug-oh0 (b1p0's av needs all jt), then ot1
        fillers += halves(vaug_part, 1, 2, 0)
        fillers += halves(vaug_part, 1, 3, 0)
        fillers += halves(proj_qk_part, 1, "wq", "qh", 1)
        fillers += halves(proj_qk_part, 1, "wk", "kh", 1)
        # duo2-4: qh/kh ot2..7
        for ot in range(2, 8):
            fillers += halves(proj_qk_part, 1, "wq", "qh", ot)
            fillers += halves(proj_qk_part, 1, "wk", "kh", ot)
        # duo5: vaug-oh1 (b1p4's av in duo6 needs it)
        for jt in range(4):
            fillers += halves(vaug_part, 1, jt, 1)
        # duo6-7: b0's output projection (b0p7's av ends in duo5)
        for it in range(4):
            for oh in range(2):
                fillers += halves(outproj_part, 0, it, oh)

        hooks = {}
        for di in range(len(duos)):
            hs = []
            if di + 2 < len(duos):
                for p in duos[di + 2]:
                    if p is not None:
                        hs.append(
                            lambda b=p["b"], hp=p["hp"]: load_pair(b, hp)
                        )
            if di == 2:
                # wm load deferred off the prologue's saturated DMA window;
                # Sync is quiet by duo2 and the deadline is duo6's outproj
                hs.append(lambda: [load_weight_half("wm", wm, h) for h in range(2)])
            hooks[di] = hs

        run_duos(duos, fillers, hooks)

        # tail: batch 1's output projection (ScalarE idle -> stores on scalar)
        for it in range(4):
            for oh in range(2):
                outproj_group(1, it, oh)

    nc.compile()
    return nc


def _get_nc():
    if "nc" not in _CACHE:
        _CACHE["nc"] = _build_nc()
    return _CACHE["nc"]


def _prep_inputs(v, k, q, img_abs, Wv, Wk, Wq, Wm, abs_mask, mask):
    import ml_dtypes

    bf16 = ml_dtypes.bfloat16
    f32 = np.float32

    def swz(x, nt):  # [B, nt*128, F] -> [B, 128, nt*F] partition-contiguous
        b, r, f = x.shape
        return np.ascontiguousarray(
            x.reshape(b, nt, 128, f).transpose(0, 2, 1, 3).reshape(b, 128, nt * f)
        )

    def t_bf(x):  # [B, L, HS] -> [B, 128, 8*L] bf16 swizzled
        xt = np.swapaxes(np.asarray(x, f32), 1, 2)
        return swz(xt, 8).astype(bf16)

    qt = t_bf(q)
    ktr = t_bf(k)
    vtr = t_bf(v)
    imt = t_bf(img_abs)

    img = np.asarray(img_abs, f32)
    augf = np.empty((B, L, H * AUGW), f32)
    augf.reshape(B, L, H, AUGW)[..., :64] = img.reshape(B, L, H, 64)
    augf.reshape(B, L, H, AUGW)[..., 64] = 1.0
    augv = swz(augf, 4).astype(bf16)

    def keepT(m):  # [B, 1, L, L] bool -> (1-m)^T swizzled bf16
        kf = 1.0 - np.asarray(m, f32)[:, 0]
        return swz(np.swapaxes(kf, 1, 2), 4).astype(bf16)

    kp1 = keepT(abs_mask)
    kp2 = keepT(mask)

    def wT(w):
        wt = np.asarray(w, f32).T  # [i, o]
        return swz(wt[None], 8)[0].astype(bf16)

    wqs, wks, wvs, wms = wT(Wq), wT(Wk), wT(Wv), wT(Wm)
    ident = np.eye(128, dtype=bf16)

    in_maps = []
    for c in range(NCORES):
        s = slice(c * BPC, (c + 1) * BPC)
        in_maps.append(
            {
                "qt": qt[s],
                "kt": ktr[s],
                "vt": vtr[s],
                "imt": imt[s],
                "aug": augv[s],
                "kp1": kp1[s],
                "kp2": kp2[s],
                "wq": wqs,
                "wk": wks,
                "wv": wvs,
                "wm": wms,
                "idt": ident,
            }
        )
    return in_maps


def kernel(v, k, q, img_abs, Wv, Wk, Wq, Wm, abs_mask, mask, _trace=False):
    _ensure_concourse()
    from concourse.bass_utils import run_bass_kernel_spmd

    in_maps = _prep_inputs(v, k, q, img_abs, Wv, Wk, Wq, Wm, abs_mask, mask)
    nc = _get_nc()
    res = run_bass_kernel_spmd(nc, in_maps, core_ids=list(range(NCORES)), trace=_trace)
    outp = np.concatenate([res.results[i]["out"] for i in range(NCORES)], axis=0)
    outp = np.asarray(outp, np.float32)  # device stores bf16; upcast on host
    if _trace:
        _CACHE["last_result"] = res
    return outp


# revision 29
# speedup vs baseline: 1.1346x; 1.0026x over previous
"""Trainium2 Bass kernel for nn_ABS_MHAtt (masked two-round multi-head attention).

Strategy: pure data-parallel over batch (B=16 -> 2 batches per NeuronCore, 8 cores,
no collectives). Host-side preprocessing (inside kernel()) pre-transposes
activations/weights into the [contraction, free] layouts the TensorEngine wants and
pre-converts everything to bf16, so the device kernel does zero layout conversion.

Per-core device kernel (per batch):
  - qhT/khT projections in transposed form [o, i]; v projected in natural form [j, o]
    directly into an "augmented" layout with a ones column per head (the ones column
    makes the PV/AV matmul also produce the softmax row-sum).
  - Per head: scores computed transposed [j, i] (contraction over d=64, head pairs
    row-tiled onto the two PE array halves), exp on ScalarE, masking by multiplying
    with (1-mask)^T (split across VectorE and GpSimdE), PV/AV with E as the
    stationary operand, one fused broadcast-multiply normalize per head, and DMA
    xbar transposes (not TensorE) to repack [i, d] tiles back to [d, i].

v3 scheduling: single interleaved stage stream. Prologue = batch 0's three
projections (dense PE while inputs land). Then 8 "duos" of attention pairs mixing
BOTH batches -- (b0p0,b0p1), (b0p2,b1p0), ..., (b1p6,b1p7) -- each duo's 8 stage
slots followed by one ~0.9us filler thunk: batch 1's projections fill the early
duos, batch 0's output projection fills the late duos, so the PE never idles long
enough for the HAM clock gate to re-throttle (attention alone is ScalarE-exp-bound
at ~8.6us/pair vs ~5.1us of PE work). imt/aug are streamed per-pair through small
rotating pools (2-duo lookahead) so both batches' working sets fit in SBUF.
ScalarE runs ONLY exp activations (plus prologue-phase evacuations and tail
stores); bulk loads ride Sync, per-pair loads ride GpSimd.
"""

import os
import sys

import numpy as np


def _ensure_concourse():
    try:
        import concourse.bass  # noqa: F401
        return
    except Exception:
        pass
    for p in ("/opt/trn_rl_repo", "/root/.axon_site/_ro/trn_rl_repo"):
        if os.path.isdir(p) and p not in sys.path:
            sys.path.insert(0, p)
            try:
                import concourse.bass  # noqa: F401
                return
            except Exception:
                sys.path.remove(p)
    raise ImportError("cannot import concourse (bass)")


B, L, HS = 16, 512, 1024
H, D = 16, 64
NCORES = 8
BPC = B // NCORES  # batches per core
SCALE = 1.0 / 8.0  # 1/sqrt(D)
AUGW = 65  # per-head augmented width (D + ones column)

_CACHE = {}


def _build_nc():
    _ensure_concourse()
    import concourse.bass as bass  # noqa: F401
    import concourse.mybir as mybir
    import concourse.tile as tile
    from concourse import bacc
    from contextlib import ExitStack

    bf = mybir.dt.bfloat16
    f32 = mybir.dt.float32
    Exp = mybir.ActivationFunctionType.Exp

    nc = bacc.Bacc()

    # all inputs host-preswizzled to [128, free] per-partition-contiguous
    # layouts so every load is one cheap 2D DMA
    qt = nc.declare_dram_parameter("qt", [BPC, 128, 8 * L], bf, isOutput=False)
    kt = nc.declare_dram_parameter("kt", [BPC, 128, 8 * L], bf, isOutput=False)
    vt = nc.declare_dram_parameter("vt", [BPC, 128, 8 * L], bf, isOutput=False)
    imt = nc.declare_dram_parameter("imt", [BPC, 128, 8 * L], bf, isOutput=False)
    aug = nc.declare_dram_parameter(
        "aug", [BPC, 128, 4 * H * AUGW], bf, isOutput=False
    )
    kp1 = nc.declare_dram_parameter("kp1", [BPC, 128, 4 * L], bf, isOutput=False)
    kp2 = nc.declare_dram_parameter("kp2", [BPC, 128, 4 * L], bf, isOutput=False)
    wq = nc.declare_dram_parameter("wq", [128, 8 * HS], bf, isOutput=False)
    wk = nc.declare_dram_parameter("wk", [128, 8 * HS], bf, isOutput=False)
    wv = nc.declare_dram_parameter("wv", [128, 8 * HS], bf, isOutput=False)
    wm = nc.declare_dram_parameter("wm", [128, 8 * HS], bf, isOutput=False)
    idt = nc.declare_dram_parameter("idt", [128, 128], bf, isOutput=False)
    out = nc.declare_dram_parameter("out", [BPC, L, HS], bf, isOutput=True)

    with ExitStack() as ctx:
        tc = ctx.enter_context(tile.TileContext(nc))
        consts = ctx.enter_context(tc.tile_pool(name="consts", bufs=1))
        inp = ctx.enter_context(tc.tile_pool(name="inp", bufs=1))
        rot = ctx.enter_context(tc.tile_pool(name="rot", bufs=6))
        proj = ctx.enter_context(tc.tile_pool(name="proj", bufs=2))
        ework = ctx.enter_context(tc.tile_pool(name="ework", bufs=2))
        small = ctx.enter_context(tc.tile_pool(name="small", bufs=3))
        evac = ctx.enter_context(tc.tile_pool(name="evac", bufs=2))
        psA = ctx.enter_context(tc.tile_pool(name="psA", bufs=3, space="PSUM"))
        psT = ctx.enter_context(tc.tile_pool(name="psT", bufs=2, space="PSUM"))

        w_sb = {}

        def load_weight_half(name, wext, half, tag=None):
            tag = tag or name
            if name not in w_sb:
                t = consts.tile([128, 8, HS], bf, tag=tag, name=name + "_sb")
                w_sb[name] = t
            t = w_sb[name]
            nc.sync.dma_start(
                out=t[:, half * 4 : (half + 1) * 4, :],
                in_=wext[:, half * 4 * HS : (half + 1) * 4 * HS],
            )

        # ---- bulk per-batch input tiles (qt/kt/vt shared between batches:
        # batch 1's load WAR-waits on batch 0's last projection reader) ----
        xin = {0: {}, 1: {}}

        def load_x_half(t, ext, b, half):
            # batch 0's bulk loads ride the ScalarE HWDGE queue (exp-idle
            # during the prologue), parallel to the weight stream on Sync;
            # batch 1's ride GpSimd so they don't crowd b0's critical path
            eng = nc.scalar if b == 0 else nc.gpsimd
            eng.dma_start(
                out=t[:, half * 4 : (half + 1) * 4, :],
                in_=ext[b][:, half * 4 * L : (half + 1) * 4 * L],
            )

        def load_bulk(b, which):
            ext = {"qt": qt, "kt": kt, "vt": vt}[which]
            t = inp.tile([128, 8, L], bf, tag=which, name=which + "_sb")
            for half in range(2):
                load_x_half(t, ext, b, half)
            xin[b][which] = t

        def load_vt_jt(b):
            # jt-sliced vt load: 4 DMAs of [128, 8, 128] so the first vaug
            # group's slice lands after ~0.25MB instead of 1MB
            t = inp.tile([128, 8, L], bf, tag="vt", name="vt_sb")
            ext = vt[b].rearrange("p (c x) -> p c x", x=L)
            eng = nc.scalar if b == 0 else nc.gpsimd
            for jt in range(4):
                eng.dma_start(
                    out=t[:, :, jt * 128 : (jt + 1) * 128],
                    in_=ext[:, :, jt * 128 : (jt + 1) * 128],
                )
            xin[b]["vt"] = t

        def load_kp(b, which):
            ext = {"kp1": kp1, "kp2": kp2}[which]
            t = inp.tile([128, 4, L], bf, tag=f"{which}_{b}", name=which + "_sb")
            nc.gpsimd.dma_start(out=t, in_=ext[b])
            xin[b][which] = t

        # ---- per-pair rotating imt/aug tiles (2-duo lookahead) ----
        pair_imt = {}
        pair_aug = {}

        def load_pair(b, hp):
            ti = rot.tile([128, L], bf, tag="imt", name="imt_p")
            nc.gpsimd.dma_start(
                out=ti, in_=imt[b][:, hp * L : (hp + 1) * L]
            )
            pair_imt[(b, hp)] = ti
            ta = rot.tile([128, 4, 2 * AUGW], bf, tag="aug", name="aug_p")
            nc.gpsimd.dma_start(
                out=ta,
                in_=aug[b].rearrange("p (jt x) -> p jt x", x=H * AUGW)[
                    :, :, 2 * hp * AUGW : (2 * hp + 2) * AUGW
                ],
            )
            pair_aug[(b, hp)] = ta

        # ---- startup: wq/qt in fine-grained interleaved chunks so the first
        # projection matmuls can start after ~400KB instead of 3MB ----
        ident = consts.tile([128, 128], bf, tag="ident")
        nc.gpsimd.dma_start(out=ident, in_=idt[:, :])
        xin[0]["qt"] = inp.tile([128, 8, L], bf, tag="qt", name="qt_sb")
        wq_t = consts.tile([128, 8, HS], bf, tag="wq", name="wq_sb")
        w_sb["wq"] = wq_t
        for c in range(4):
            nc.sync.dma_start(
                out=wq_t[:, 2 * c : 2 * c + 2, :],
                in_=wq[:, 2 * c * HS : (2 * c + 2) * HS],
            )
            nc.scalar.dma_start(
                out=xin[0]["qt"][:, 2 * c : 2 * c + 2, :],
                in_=qt[0][:, 2 * c * L : (2 * c + 2) * L],
            )
        # HAM warm-up: ~3.5us of back-to-back dummy transposes (gated only on
        # the tiny ident load) promote the PE clock gate to 2.4 GHz during the
        # DMA-bound startup, instead of ramping mid-prologue
        for w in range(64):
            wps = psT.tile([128, 128], bf, tag="tail", name="warm")
            nc.tensor.transpose(wps, ident, ident)

        # ---- per-batch working tiles ----
        st = {}
        for b in range(BPC):
            st[b] = {
                "qh": proj.tile([128, 8, L], bf, tag="qh", name="qh_sb"),
                "kh": proj.tile([128, 8, L], bf, tag="kh", name="kh_sb"),
                "vaug": proj.tile(
                    [128, 4, H * AUGW], bf, tag="vaug", name="vaug_sb"
                ),
                "att": proj.tile([128, 8, L], bf, tag="att", name="att_sb"),
            }

        # ---- projection groups (one PSUM group each; interleavable thunks) ----
        def proj_qk_part(b, wname, dstname, ot, part, state):
            """half of a projection PSUM group (~0.9us of PE work)."""
            wt = w_sb[wname]
            xsb = xin[b]["qt" if wname == "wq" else "kt"]
            dst = st[b][dstname]
            if part == 0:
                state["ps"] = psA.tile([128, 512], f32, tag="psA", name="ps_pj")
            ps = state["ps"]
            for kc in range(4 * part, 4 * part + 4):
                nc.tensor.matmul(
                    ps,
                    wt[:, kc, ot * 128 : (ot + 1) * 128],
                    xsb[:, kc, :],
                    start=(kc == 0),
                    stop=(kc == 7),
                )
            if part == 1:
                if b == 0:
                    # b0's evacs ride ScalarE (exp-idle during the prologue);
                    # b1's ride DVE (GpSimd has no PSUM port)
                    nc.scalar.copy(out=dst[:, ot, :], in_=ps)
                else:
                    nc.vector.tensor_copy(out=dst[:, ot, :], in_=ps)

        def proj_qk_group(b, wname, dstname, ot):
            state = {}
            proj_qk_part(b, wname, dstname, ot, 0, state)
            proj_qk_part(b, wname, dstname, ot, 1, state)

        def vaug_part(b, jt, oh, part, state):
            """half of a v-projection group into the augmented layout."""
            vaug_sb = st[b]["vaug"]
            vt_sb = xin[b]["vt"]
            if part == 0 and oh == 0:
                nc.vector.memset(
                    vaug_sb[:, jt, :].rearrange("p (h x) -> p h x", x=AUGW)[
                        :, :, 64
                    ],
                    1.0,
                )
            if part == 0:
                state["ps"] = psA.tile([128, 512], f32, tag="psA", name="ps_pj")
            ps = state["ps"]
            for kc in range(4 * part, 4 * part + 4):
                nc.tensor.matmul(
                    ps,
                    vt_sb[:, kc, jt * 128 : (jt + 1) * 128],
                    w_sb["wv"][:, kc, oh * 512 : (oh + 1) * 512],
                    start=(kc == 0),
                    stop=(kc == 7),
                )
            if part == 1:
                dst_ap = vaug_sb[
                    :, jt, oh * 8 * AUGW : (oh + 1) * 8 * AUGW
                ].rearrange("p (h x) -> p h x", x=AUGW)[:, :, 0:64]
                nc.vector.tensor_copy(
                    out=dst_ap, in_=ps.rearrange("p (h x) -> p h x", x=64)
                )

        def vaug_group(b, jt, oh):
            state = {}
            vaug_part(b, jt, oh, 0, state)
            vaug_part(b, jt, oh, 1, state)

        def outproj_part(b, it, oh, part, state):
            """half of an output-projection PSUM group (~0.9us of PE work)."""
            att_sb = st[b]["att"]
            if part == 0:
                state["ps"] = psA.tile([128, 512], f32, tag="psA", name="ps_pj")
            ps = state["ps"]
            for kc in range(4 * part, 4 * part + 4):
                nc.tensor.matmul(
                    ps,
                    att_sb[:, kc, it * 128 : (it + 1) * 128],
                    w_sb["wm"][:, kc, oh * 512 : (oh + 1) * 512],
                    start=(kc == 0),
                    stop=(kc == 7),
                )
            if part == 1:
                ob = evac.tile([128, 512], bf, tag="ob")
                g = 2 * it + oh
                if b == 1:
                    # tail: exp is done -- alternate evacs across ScalarE and
                    # DVE, and stores across three DMA queues, so the 8
                    # groups' drains run as parallel chains instead of one
                    if g % 2 == 0:
                        nc.scalar.copy(out=ob, in_=ps)
                    else:
                        nc.vector.tensor_copy(out=ob, in_=ps)
                    eng = (nc.scalar, nc.gpsimd, nc.sync)[g % 3]
                else:
                    nc.vector.tensor_copy(out=ob, in_=ps)
                    eng = nc.sync
                eng.dma_start(
                    out=out[
                        b, it * 128 : (it + 1) * 128, oh * 512 : (oh + 1) * 512
                    ],
                    in_=ob,
                )

        def outproj_group(b, it, oh):
            state = {}
            outproj_part(b, it, oh, 0, state)
            outproj_part(b, it, oh, 1, state)

        # ---- attention stages ----
        def score_stage(b, hp, lhs_fn, rhs_fn, etile):
            """s^T [j,i] for both heads of pair hp + exp into etile.

            The two heads' matmuls use lhsT base partitions 0 / 64, so they
            run concurrently on the two row-halves of the PE array (outputs
            land in different PSUM banks)."""
            for jt in range(4):
                ps = psA.tile([128, 1024], f32, tag="psA", name="ps_sc")
                for g in range(2):
                    nc.tensor.matmul(
                        ps[:, g * 512 : (g + 1) * 512],
                        lhs_fn(g, jt),
                        rhs_fn(g),
                        start=True,
                        stop=True,
                    )
                nc.scalar.activation(
                    out=etile[:, jt],
                    in_=ps.rearrange("p (g x) -> p g x", x=512),
                    func=Exp,
                    scale=SCALE,
                )

        def mask_stage(b, hp, kp_sb, etile):
            # in-place mask multiply, one fused op per jt covering both heads
            # (mask row broadcast across the head dim via a stride-0 AP).
            # All on DVE: a GpSimd-offloaded op costs ~2.2us on the
            # exp->mask->pv critical chain and stalls the in-order PE queue.
            for jt in range(4):
                kpb = kp_sb[:, jt, :].unsqueeze(1).broadcast_to([128, 2, L])
                nc.vector.tensor_mul(etile[:, jt], etile[:, jt], kpb)

        def pv_stage(b, hp, emtile, rhs_fn):
            """pv natural [i, 4*65] per head -> normalized dl pair [128,4,128].

            One fused broadcast-multiply per head turns the raw PSUM pv tile
            into the normalized bf16 dl tile (recip row-sums broadcast along
            d via a stride-0 AP)."""
            dl = small.tile([128, 4, 128], bf, tag="dl")
            for g in range(2):
                pspv = psT.tile([128, 4, AUGW], f32, tag="tail")
                for it in range(4):
                    for jt in range(4):
                        nc.tensor.matmul(
                            pspv[:, it, :],
                            emtile[:, jt, g, it * 128 : (it + 1) * 128],
                            rhs_fn(g, jt),
                            start=(jt == 0),
                            stop=(jt == 3),
                        )
                r1 = small.tile([128, 4], f32, tag="r1")
                nc.vector.reciprocal(r1, pspv[:, :, 64])
                nc.vector.tensor_mul(
                    dl[:, :, g * 64 : (g + 1) * 64],
                    pspv[:, :, 0:64],
                    r1.unsqueeze(-1).broadcast_to([128, 4, 64]),
                )
            return dl

        def s1_stage(b, hp):
            e1 = ework.tile([128, 4, 2, L], bf, tag="e")
            imt_t = pair_imt[(b, hp)]
            qh_sb = st[b]["qh"]
            score_stage(
                b,
                hp,
                lambda g, jt: imt_t[g * 64 : g * 64 + 64, jt * 128 : (jt + 1) * 128],
                lambda g: qh_sb[g * 64 : g * 64 + 64, hp, :],
                e1,
            )
            mask_stage(b, hp, xin[b]["kp1"], e1)
            return e1

        def mod_stage(b, hp, e1):
            """round-1 tail: pv + normalize + PE-transpose + add qh -> qn_pair.

            This transpose is on the s2 latency chain, so it stays on TensorE
            instead of the ~1.2us-per-op DMA xbar path."""
            aug_t = pair_aug[(b, hp)]
            dl = pv_stage(
                b, hp, e1,
                lambda g, jt: aug_t[:, jt, g * AUGW : (g + 1) * AUGW],
            )
            pst = psT.tile([128, 512], bf, tag="tail", name="pst")
            for it in range(4):
                nc.tensor.transpose(
                    pst[:, it * 128 : (it + 1) * 128], dl[:, it, :], ident
                )
            qn_pair = small.tile([128, 512], bf, tag="qnp")
            nc.vector.tensor_add(qn_pair, pst, st[b]["qh"][:, hp, :])
            return qn_pair

        def s2_stage(b, hp, qn_pair):
            e2 = ework.tile([128, 4, 2, L], bf, tag="e")
            kh_sb = st[b]["kh"]
            score_stage(
                b,
                hp,
                lambda g, jt: kh_sb[g * 64 : g * 64 + 64, hp, jt * 128 : (jt + 1) * 128],
                lambda g: qn_pair[g * 64 : g * 64 + 64, :],
                e2,
            )
            mask_stage(b, hp, xin[b]["kp2"], e2)
            return e2

        def av_stage(b, hp, e2):
            """round-2 tail: av + normalize + transpose -> att[:, hp, :].

            Normally via DMA xbar (off the engines); the LAST duo's pairs use
            TensorE transposes instead -- their repack gates the tail output
            projection, and the xbar path costs ~1.2us per tile plus sync-
            queue waits right when the tail needs att complete."""
            vaug_sb = st[b]["vaug"]
            dl = pv_stage(
                b, hp, e2,
                lambda g, jt: vaug_sb[:, jt, (2 * hp + g) * AUGW : (2 * hp + g + 1) * AUGW],
            )
            if b == 1 and hp >= 6:
                pst = psT.tile([128, 512], bf, tag="tail", name="pst")
                for it in range(4):
                    nc.tensor.transpose(
                        pst[:, it * 128 : (it + 1) * 128], dl[:, it, :], ident
                    )
                nc.vector.tensor_copy(out=st[b]["att"][:, hp, :], in_=pst)
            else:
                for it in range(4):
                    nc.sync.dma_start_transpose(
                        out=st[b]["att"][:, hp, it * 128 : (it + 1) * 128],
                        in_=dl[:, it, :],
                    )

        # ---- stage-stream runner ----
        def do_s1(p):
            p["e1"] = s1_stage(p["b"], p["hp"])

        def do_mod(p):
            p["qn"] = mod_stage(p["b"], p["hp"], p["e1"])

        def do_s2(p):
            p["e2"] = s2_stage(p["b"], p["hp"], p["qn"])

        def do_av(p):
            av_stage(p["b"], p["hp"], p["e2"])

        def run_duos(duos, fillers, preduo_hooks):
            for di, (pa, pb) in enumerate(duos):
                for hook in preduo_hooks.get(di, ()):
                    hook()
                for fn, p in (
                    (do_s1, pa), (do_s1, pb),
                    (do_mod, pa), (do_mod, pb),
                    (do_s2, pa), (do_s2, pb),
                    (do_av, pa), (do_av, pb),
                ):
                    if p is not None:
                        fn(p)
                    if fillers:
                        t = fillers.pop(0)
                        if t is not None:
                            t()
            while fillers:
                t = fillers.pop(0)
                if t is not None:
                    t()

        # ---- schedule ----
        # prologue: batch 0's three projections, dense PE while inputs land.
        # qh first (so qt(b1)'s WAR clears early), then kh, then vaug.
        # Weight stream on Sync; activation bulk loads on DVE's queue
        # (parallel DMA queues halve the prologue's DMA critical path);
        # per-pair imt/aug + keep-masks on GpSimd's queue.
        load_kp(0, "kp1")
        load_kp(0, "kp2")
        # imt/aug for duo0+duo1's pairs (hooks cover duo2 onward)
        for p_ in ((0, 0), (0, 1), (0, 2), (1, 0)):
            load_pair(*p_)
        kt0 = inp.tile([128, 8, L], bf, tag="kt", name="kt_sb")
        xin[0]["kt"] = kt0
        vt0 = inp.tile([128, 8, L], bf, tag="vt", name="vt_sb")
        xin[0]["vt"] = vt0
        for ot in range(8):
            if ot < 2:
                load_weight_half("wk", wk, ot)
                load_x_half(kt0, kt, 0, ot)
            elif ot < 4:
                load_weight_half("wv", wv, ot - 2)
                load_x_half(vt0, vt, 0, ot - 2)
            proj_qk_group(0, "wq", "qh", ot)
        load_bulk(1, "qt")  # WAR on b0's qh groups just cleared
        load_kp(1, "kp1")
        for ot in range(8):
            proj_qk_group(0, "wk", "kh", ot)
        load_bulk(1, "kt")
        load_kp(1, "kp2")
        for jt in range(4):
            for oh in range(2):
                vaug_group(0, jt, oh)
        load_vt_jt(1)

        P = {}
        for b in range(2):
            for hp in range(8):
                P[(b, hp)] = {"b": b, "hp": hp}

        duos = [
            (P[(0, 0)], P[(0, 1)]),
            (P[(0, 2)], P[(1, 0)]),
            (P[(0, 3)], P[(1, 1)]),
            (P[(0, 4)], P[(1, 2)]),
            (P[(0, 5)], P[(1, 3)]),
            (P[(0, 6)], P[(0, 7)]),
            (P[(1, 4)], P[(1, 5)]),
            (P[(1, 6)], P[(1, 7)]),
        ]

        def halves(fn, *args):
            state = {}
            return [
                (lambda part=part, state=state: fn(*args, part, state))
                for part in range(2)
            ]

        fillers = []
        # duo0: b1's first qh/kh groups + first half of vaug-oh0
        fillers += halves(proj_qk_part, 1, "wq", "qh", 0)
        fillers += halves(proj_qk_part, 1, "wk", "kh", 0)
        fillers += halves(vaug_part, 1, 0, 0)
        fillers += halves(vaug_part, 1, 1, 0)
        # duo1: rest of vaug-oh0 (b1p0's av needs all jt), then ot1
        fillers += halves(vaug_part, 1, 2, 0)
        fillers += halves(vaug_part, 1, 3, 0)
        fillers += halves(proj_qk_part, 1, "wq", "qh", 1)
        fillers += halves(proj_qk_part, 1, "wk", "kh", 1)
        # duo2-4: qh/kh ot2..7
        for ot in range(2, 8):
            fillers += halves(proj_qk_part, 1, "wq", "qh", ot)
            fillers += halves(proj_qk_part, 1, "wk", "kh", ot)
        # duo5: vaug-oh1 (b1p4's av in duo6 needs it)
        for jt in range(4):
            fillers += halves(vaug_part, 1, jt, 1)
        # duo6: b0's output projection groups 0-3 (b0p7's av ends in duo5)
        for it in range(2):
            for oh in range(2):
                fillers += halves(outproj_part, 0, it, oh)
        # duo7: b0's groups 4-6 as full-slot fillers, g7 split, then
        # PRE-START batch 1's first three output groups' kc0-3 accumulations
        # (their att inputs finished back in duo4; nothing else needs psA
        # after duo7's last score, so holding 3 psA bufs into the tail is
        # conflict-free and shaves ~3us off the tail)
        fillers.append(lambda: outproj_group(0, 2, 0))
        fillers.append(lambda: outproj_group(0, 2, 1))
        fillers.append(lambda: outproj_group(0, 3, 0))
        fillers += halves(outproj_part, 0, 3, 1)
        b1_pre = {(0, 0): {}, (0, 1): {}, (1, 0): {}}
        for (it, oh), stt in b1_pre.items():
            fillers.append(
                lambda it=it, oh=oh, stt=stt: outproj_part(1, it, oh, 0, stt)
            )

        hooks = {}
        for di in range(len(duos)):
            hs = []
            if di + 2 < len(duos):
                for p in duos[di + 2]:
                    if p is not None:
                        hs.append(
                            lambda b=p["b"], hp=p["hp"]: load_pair(b, hp)
                        )
            if di == 2:
                # wm load deferred off the prologue's saturated DMA window;
                # Sync is quiet by duo2 and the deadline is duo6's outproj
                hs.append(lambda: [load_weight_half("wm", wm, h) for h in range(2)])
            hooks[di] = hs

        run_duos(duos, fillers, hooks)

        # tail: batch 1's output projection (pre-started groups finish first)
        for it in range(4):
            for oh in range(2):
                if (it, oh) in b1_pre:
                    outproj_part(1, it, oh, 1, b1_pre[(it, oh)])
                else:
                    outproj_group(1, it, oh)

    nc.compile()
    return nc


def _get_nc():
    if "nc" not in _CACHE:
        _CACHE["nc"] = _build_nc()
    return _CACHE["nc"]


def _prep_inputs(v, k, q, img_abs, Wv, Wk, Wq, Wm, abs_mask, mask):
    import ml_dtypes

    bf16 = ml_dtypes.bfloat16
    f32 = np.float32

    def swz(x, nt):  # [B, nt*128, F] -> [B, 128, nt*F] partition-contiguous
        b, r, f = x.shape
        return np.ascontiguousarray(
            x.reshape(b, nt, 128, f).transpose(0, 2, 1, 3).reshape(b, 128, nt * f)
        )

    def t_bf(x):  # [B, L, HS] -> [B, 128, 8*L] bf16 swizzled
        xt = np.swapaxes(np.asarray(x, f32), 1, 2)
        return swz(xt, 8).astype(bf16)

    qt = t_bf(q)
    ktr = t_bf(k)
    vtr = t_bf(v)
    imt = t_bf(img_abs)

    img = np.asarray(img_abs, f32)
    augf = np.empty((B, L, H * AUGW), f32)
    augf.reshape(B, L, H, AUGW)[..., :64] = img.reshape(B, L, H, 64)
    augf.reshape(B, L, H, AUGW)[..., 64] = 1.0
    augv = swz(augf, 4).astype(bf16)

    def keepT(m):  # [B, 1, L, L] bool -> (1-m)^T swizzled bf16
        kf = 1.0 - np.asarray(m, f32)[:, 0]
        return swz(np.swapaxes(kf, 1, 2), 4).astype(bf16)

    kp1 = keepT(abs_mask)
    kp2 = keepT(mask)

    def wT(w):
        wt = np.asarray(w, f32).T  # [i, o]
        return swz(wt[None], 8)[0].astype(bf16)

    wqs, wks, wvs, wms = wT(Wq), wT(Wk), wT(Wv), wT(Wm)
    ident = np.eye(128, dtype=bf16)

    in_maps = []
    for c in range(NCORES):
        s = slice(c * BPC, (c + 1) * BPC)
        in_maps.append(
            {
                "qt": qt[s],
                "kt": ktr[s],
                "vt": vtr[s],
                "imt": imt[s],
                "aug": augv[s],
                "kp1": kp1[s],
                "kp2": kp2[s],
                "wq": wqs,
                "wk": wks,
                "wv": wvs,
                "wm": wms,
                "idt": ident,
            }
        )
    return in_maps


def kernel(v, k, q, img_abs, Wv, Wk, Wq, Wm, abs_mask, mask, _trace=False):
    _ensure_concourse()
    from concourse.bass_utils import run_bass_kernel_spmd

    in_maps = _prep_inputs(v, k, q, img_abs, Wv, Wk, Wq, Wm, abs_mask, mask)
    nc = _get_nc()
    res = run_bass_kernel_spmd(nc, in_maps, core_ids=list(range(NCORES)), trace=_trace)
    outp = np.concatenate([res.results[i]["out"] for i in range(NCORES)], axis=0)
    outp = np.asarray(outp, np.float32)  # device stores bf16; upcast on host
    if _trace:
        _CACHE["last_result"] = res
    return outp


# revision 33
# speedup vs baseline: 1.1397x; 1.0045x over previous
"""Trainium2 Bass kernel for nn_ABS_MHAtt (masked two-round multi-head attention).

Strategy: pure data-parallel over batch (B=16 -> 2 batches per NeuronCore, 8 cores,
no collectives). Host-side preprocessing (inside kernel()) pre-transposes
activations/weights into the [contraction, free] layouts the TensorEngine wants and
pre-converts everything to bf16, so the device kernel does zero layout conversion.

Per-core device kernel (per batch):
  - qhT/khT projections in transposed form [o, i]; v projected in natural form [j, o]
    directly into an "augmented" layout with a ones column per head (the ones column
    makes the PV/AV matmul also produce the softmax row-sum).
  - Per head: scores computed transposed [j, i] (contraction over d=64, head pairs
    row-tiled onto the two PE array halves), exp on ScalarE, masking by multiplying
    with (1-mask)^T (split across VectorE and GpSimdE), PV/AV with E as the
    stationary operand, one fused broadcast-multiply normalize per head, and DMA
    xbar transposes (not TensorE) to repack [i, d] tiles back to [d, i].

v3 scheduling: single interleaved stage stream. Prologue = batch 0's three
projections (dense PE while inputs land). Then 8 "duos" of attention pairs mixing
BOTH batches -- (b0p0,b0p1), (b0p2,b1p0), ..., (b1p6,b1p7) -- each duo's 8 stage
slots followed by one ~0.9us filler thunk: batch 1's projections fill the early
duos, batch 0's output projection fills the late duos, so the PE never idles long
enough for the HAM clock gate to re-throttle (attention alone is ScalarE-exp-bound
at ~8.6us/pair vs ~5.1us of PE work). imt/aug are streamed per-pair through small
rotating pools (2-duo lookahead) so both batches' working sets fit in SBUF.
ScalarE runs ONLY exp activations (plus prologue-phase evacuations and tail
stores); bulk loads ride Sync, per-pair loads ride GpSimd.
"""

import os
import sys

import numpy as np


def _ensure_concourse():
    try:
        import concourse.bass  # noqa: F401
        return
    except Exception:
        pass
    for p in ("/opt/trn_rl_repo", "/root/.axon_site/_ro/trn_rl_repo"):
        if os.path.isdir(p) and p not in sys.path:
            sys.path.insert(0, p)
            try:
                import concourse.bass  # noqa: F401
                return
            except Exception:
                sys.path.remove(p)
    raise ImportError("cannot import concourse (bass)")


B, L, HS = 16, 512, 1024
H, D = 16, 64
NCORES = 8
BPC = B // NCORES  # batches per core
SCALE = 1.0 / 8.0  # 1/sqrt(D)
AUGW = 65  # per-head augmented width (D + ones column)

_CACHE = {}


def _build_nc():
    _ensure_concourse()
    import concourse.bass as bass  # noqa: F401
    import concourse.mybir as mybir
    import concourse.tile as tile
    from concourse import bacc
    from contextlib import ExitStack

    bf = mybir.dt.bfloat16
    f32 = mybir.dt.float32
    Exp = mybir.ActivationFunctionType.Exp

    nc = bacc.Bacc()

    # all inputs host-preswizzled to [128, free] per-partition-contiguous
    # layouts so every load is one cheap 2D DMA
    qt = nc.declare_dram_parameter("qt", [BPC, 128, 8 * L], bf, isOutput=False)
    kt = nc.declare_dram_parameter("kt", [BPC, 128, 8 * L], bf, isOutput=False)
    vt = nc.declare_dram_parameter("vt", [BPC, 128, 8 * L], bf, isOutput=False)
    imt = nc.declare_dram_parameter("imt", [BPC, 128, 8 * L], bf, isOutput=False)
    aug = nc.declare_dram_parameter(
        "aug", [BPC, 128, 4 * H * AUGW], bf, isOutput=False
    )
    kp1 = nc.declare_dram_parameter("kp1", [BPC, 128, 4 * L], bf, isOutput=False)
    kp2 = nc.declare_dram_parameter("kp2", [BPC, 128, 4 * L], bf, isOutput=False)
    wq = nc.declare_dram_parameter("wq", [128, 8 * HS], bf, isOutput=False)
    wk = nc.declare_dram_parameter("wk", [128, 8 * HS], bf, isOutput=False)
    wv = nc.declare_dram_parameter("wv", [128, 8 * HS], bf, isOutput=False)
    wm = nc.declare_dram_parameter("wm", [128, 8 * HS], bf, isOutput=False)
    idt = nc.declare_dram_parameter("idt", [128, 128], bf, isOutput=False)
    out = nc.declare_dram_parameter("out", [BPC, L, HS], bf, isOutput=True)

    with ExitStack() as ctx:
        tc = ctx.enter_context(tile.TileContext(nc))
        consts = ctx.enter_context(tc.tile_pool(name="consts", bufs=1))
        inp = ctx.enter_context(tc.tile_pool(name="inp", bufs=1))
        rot = ctx.enter_context(tc.tile_pool(name="rot", bufs=6))
        proj = ctx.enter_context(tc.tile_pool(name="proj", bufs=2))
        ework = ctx.enter_context(tc.tile_pool(name="ework", bufs=2))
        small = ctx.enter_context(tc.tile_pool(name="small", bufs=3))
        evac = ctx.enter_context(tc.tile_pool(name="evac", bufs=2))
        psA = ctx.enter_context(tc.tile_pool(name="psA", bufs=3, space="PSUM"))
        psT = ctx.enter_context(tc.tile_pool(name="psT", bufs=2, space="PSUM"))

        w_sb = {}

        def load_weight_half(name, wext, half, tag=None):
            tag = tag or name
            if name not in w_sb:
                t = consts.tile([128, 8, HS], bf, tag=tag, name=name + "_sb")
                w_sb[name] = t
            t = w_sb[name]
            nc.sync.dma_start(
                out=t[:, half * 4 : (half + 1) * 4, :],
                in_=wext[:, half * 4 * HS : (half + 1) * 4 * HS],
            )

        # ---- bulk per-batch input tiles (qt/kt/vt shared between batches:
        # batch 1's load WAR-waits on batch 0's last projection reader) ----
        xin = {0: {}, 1: {}}

        def load_x_half(t, ext, b, half):
            # batch 0's bulk loads ride the ScalarE HWDGE queue (exp-idle
            # during the prologue), parallel to the weight stream on Sync;
            # batch 1's ride GpSimd so they don't crowd b0's critical path
            eng = nc.scalar if b == 0 else nc.gpsimd
            eng.dma_start(
                out=t[:, half * 4 : (half + 1) * 4, :],
                in_=ext[b][:, half * 4 * L : (half + 1) * 4 * L],
            )

        def load_bulk(b, which):
            ext = {"qt": qt, "kt": kt, "vt": vt}[which]
            t = inp.tile([128, 8, L], bf, tag=which, name=which + "_sb")
            for half in range(2):
                load_x_half(t, ext, b, half)
            xin[b][which] = t

        def load_vt_jt(b):
            # jt-sliced vt load: 4 DMAs of [128, 8, 128] so the first vaug
            # group's slice lands after ~0.25MB instead of 1MB
            t = inp.tile([128, 8, L], bf, tag="vt", name="vt_sb")
            ext = vt[b].rearrange("p (c x) -> p c x", x=L)
            eng = nc.scalar if b == 0 else nc.gpsimd
            for jt in range(4):
                eng.dma_start(
                    out=t[:, :, jt * 128 : (jt + 1) * 128],
                    in_=ext[:, :, jt * 128 : (jt + 1) * 128],
                )
            xin[b]["vt"] = t

        def load_kp(b, which):
            ext = {"kp1": kp1, "kp2": kp2}[which]
            t = inp.tile([128, 4, L], bf, tag=f"{which}_{b}", name=which + "_sb")
            nc.gpsimd.dma_start(out=t, in_=ext[b])
            xin[b][which] = t

        # ---- per-pair rotating imt/aug tiles (2-duo lookahead) ----
        pair_imt = {}
        pair_aug = {}

        def load_pair(b, hp):
            ti = rot.tile([128, L], bf, tag="imt", name="imt_p")
            nc.gpsimd.dma_start(
                out=ti, in_=imt[b][:, hp * L : (hp + 1) * L]
            )
            pair_imt[(b, hp)] = ti
            ta = rot.tile([128, 4, 2 * AUGW], bf, tag="aug", name="aug_p")
            nc.gpsimd.dma_start(
                out=ta,
                in_=aug[b].rearrange("p (jt x) -> p jt x", x=H * AUGW)[
                    :, :, 2 * hp * AUGW : (2 * hp + 2) * AUGW
                ],
            )
            pair_aug[(b, hp)] = ta

        # ---- startup: wq/qt in fine-grained interleaved chunks so the first
        # projection matmuls can start after ~400KB instead of 3MB ----
        ident = consts.tile([128, 128], bf, tag="ident")
        nc.gpsimd.dma_start(out=ident, in_=idt[:, :])
        xin[0]["qt"] = inp.tile([128, 8, L], bf, tag="qt", name="qt_sb")
        wq_t = consts.tile([128, 8, HS], bf, tag="wq", name="wq_sb")
        w_sb["wq"] = wq_t
        for c in range(4):
            nc.sync.dma_start(
                out=wq_t[:, 2 * c : 2 * c + 2, :],
                in_=wq[:, 2 * c * HS : (2 * c + 2) * HS],
            )
            nc.scalar.dma_start(
                out=xin[0]["qt"][:, 2 * c : 2 * c + 2, :],
                in_=qt[0][:, 2 * c * L : (2 * c + 2) * L],
            )
        # HAM warm-up: ~3.5us of back-to-back dummy transposes (gated only on
        # the tiny ident load) promote the PE clock gate to 2.4 GHz during the
        # DMA-bound startup, instead of ramping mid-prologue
        for w in range(64):
            wps = psT.tile([128, 128], bf, tag="tail", name="warm")
            nc.tensor.transpose(wps, ident, ident)

        # ---- per-batch working tiles ----
        st = {}
        for b in range(BPC):
            st[b] = {
                "qh": proj.tile([128, 8, L], bf, tag="qh", name="qh_sb"),
                "kh": proj.tile([128, 8, L], bf, tag="kh", name="kh_sb"),
                "vaug": proj.tile(
                    [128, 4, H * AUGW], bf, tag="vaug", name="vaug_sb"
                ),
                "att": proj.tile([128, 8, L], bf, tag="att", name="att_sb"),
            }

        # ---- projection groups (one PSUM group each; interleavable thunks) ----
        def proj_qk_part(b, wname, dstname, ot, part, state):
            """half of a projection PSUM group (~0.9us of PE work)."""
            wt = w_sb[wname]
            xsb = xin[b]["qt" if wname == "wq" else "kt"]
            dst = st[b][dstname]
            if part == 0:
                state["ps"] = psA.tile([128, 512], f32, tag="psA", name="ps_pj")
            ps = state["ps"]
            for kc in range(4 * part, 4 * part + 4):
                nc.tensor.matmul(
                    ps,
                    wt[:, kc, ot * 128 : (ot + 1) * 128],
                    xsb[:, kc, :],
                    start=(kc == 0),
                    stop=(kc == 7),
                )
            if part == 1:
                if b == 0:
                    # b0's evacs ride ScalarE (exp-idle during the prologue);
                    # b1's ride DVE (GpSimd has no PSUM port)
                    nc.scalar.copy(out=dst[:, ot, :], in_=ps)
                else:
                    nc.vector.tensor_copy(out=dst[:, ot, :], in_=ps)

        def proj_qk_group(b, wname, dstname, ot):
            state = {}
            proj_qk_part(b, wname, dstname, ot, 0, state)
            proj_qk_part(b, wname, dstname, ot, 1, state)

        def vaug_part(b, jt, oh, part, state):
            """half of a v-projection group into the augmented layout."""
            vaug_sb = st[b]["vaug"]
            vt_sb = xin[b]["vt"]
            if part == 0 and oh == 0:
                nc.vector.memset(
                    vaug_sb[:, jt, :].rearrange("p (h x) -> p h x", x=AUGW)[
                        :, :, 64
                    ],
                    1.0,
                )
            if part == 0:
                state["ps"] = psA.tile([128, 512], f32, tag="psA", name="ps_pj")
            ps = state["ps"]
            for kc in range(4 * part, 4 * part + 4):
                nc.tensor.matmul(
                    ps,
                    vt_sb[:, kc, jt * 128 : (jt + 1) * 128],
                    w_sb["wv"][:, kc, oh * 512 : (oh + 1) * 512],
                    start=(kc == 0),
                    stop=(kc == 7),
                )
            if part == 1:
                dst_ap = vaug_sb[
                    :, jt, oh * 8 * AUGW : (oh + 1) * 8 * AUGW
                ].rearrange("p (h x) -> p h x", x=AUGW)[:, :, 0:64]
                nc.vector.tensor_copy(
                    out=dst_ap, in_=ps.rearrange("p (h x) -> p h x", x=64)
                )

        def vaug_group(b, jt, oh):
            state = {}
            vaug_part(b, jt, oh, 0, state)
            vaug_part(b, jt, oh, 1, state)

        def outproj_part(b, it, oh, part, state):
            """half of an output-projection PSUM group (~0.9us of PE work)."""
            att_sb = st[b]["att"]
            if part == 0:
                state["ps"] = psA.tile([128, 512], f32, tag="psA", name="ps_pj")
            ps = state["ps"]
            for kc in range(4 * part, 4 * part + 4):
                nc.tensor.matmul(
                    ps,
                    att_sb[:, kc, it * 128 : (it + 1) * 128],
                    w_sb["wm"][:, kc, oh * 512 : (oh + 1) * 512],
                    start=(kc == 0),
                    stop=(kc == 7),
                )
            if part == 1:
                ob = evac.tile([128, 512], bf, tag="ob")
                g = 2 * it + oh
                if b == 1:
                    # tail: exp is done -- alternate evacs across ScalarE and
                    # DVE, and stores across three DMA queues, so the 8
                    # groups' drains run as parallel chains instead of one
                    if g % 2 == 0:
                        nc.scalar.copy(out=ob, in_=ps)
                    else:
                        nc.vector.tensor_copy(out=ob, in_=ps)
                    eng = (nc.scalar, nc.gpsimd, nc.sync)[g % 3]
                else:
                    nc.vector.tensor_copy(out=ob, in_=ps)
                    eng = nc.sync
                eng.dma_start(
                    out=out[
                        b, it * 128 : (it + 1) * 128, oh * 512 : (oh + 1) * 512
                    ],
                    in_=ob,
                )

        def outproj_group(b, it, oh):
            state = {}
            outproj_part(b, it, oh, 0, state)
            outproj_part(b, it, oh, 1, state)

        # ---- attention stages ----
        def score_stage(b, hp, lhs_fn, rhs_fn, etile):
            """s^T [j,i] for both heads of pair hp + exp into etile.

            The two heads' matmuls use lhsT base partitions 0 / 64, so they
            run concurrently on the two row-halves of the PE array (outputs
            land in different PSUM banks)."""
            for jt in range(4):
                ps = psA.tile([128, 1024], f32, tag="psA", name="ps_sc")
                for g in range(2):
                    nc.tensor.matmul(
                        ps[:, g * 512 : (g + 1) * 512],
                        lhs_fn(g, jt),
                        rhs_fn(g),
                        start=True,
                        stop=True,
                    )
                nc.scalar.activation(
                    out=etile[:, jt],
                    in_=ps.rearrange("p (g x) -> p g x", x=512),
                    func=Exp,
                    scale=SCALE,
                )

        def mask_stage(b, hp, kp_sb, etile):
            # in-place mask multiply, one fused op per jt covering both heads
            # (mask row broadcast across the head dim via a stride-0 AP).
            # All on DVE: a GpSimd-offloaded op costs ~2.2us on the
            # exp->mask->pv critical chain and stalls the in-order PE queue.
            for jt in range(4):
                kpb = kp_sb[:, jt, :].unsqueeze(1).broadcast_to([128, 2, L])
                nc.vector.tensor_mul(etile[:, jt], etile[:, jt], kpb)

        def pv_stage(b, hp, emtile, rhs_fn):
            """pv natural [i, 4*65] per head -> normalized dl pair [128,4,128].

            One fused broadcast-multiply per head turns the raw PSUM pv tile
            into the normalized bf16 dl tile (recip row-sums broadcast along
            d via a stride-0 AP)."""
            dl = small.tile([128, 4, 128], bf, tag="dl")
            for g in range(2):
                pspv = psT.tile([128, 4, AUGW], f32, tag="tail")
                for it in range(4):
                    for jt in range(4):
                        nc.tensor.matmul(
                            pspv[:, it, :],
                            emtile[:, jt, g, it * 128 : (it + 1) * 128],
                            rhs_fn(g, jt),
                            start=(jt == 0),
                            stop=(jt == 3),
                        )
                r1 = small.tile([128, 4], f32, tag="r1")
                nc.vector.reciprocal(r1, pspv[:, :, 64])
                nc.vector.tensor_mul(
                    dl[:, :, g * 64 : (g + 1) * 64],
                    pspv[:, :, 0:64],
                    r1.unsqueeze(-1).broadcast_to([128, 4, 64]),
                )
            return dl

        def s1_stage(b, hp):
            e1 = ework.tile([128, 4, 2, L], bf, tag="e")
            imt_t = pair_imt[(b, hp)]
            qh_sb = st[b]["qh"]
            score_stage(
                b,
                hp,
                lambda g, jt: imt_t[g * 64 : g * 64 + 64, jt * 128 : (jt + 1) * 128],
                lambda g: qh_sb[g * 64 : g * 64 + 64, hp, :],
                e1,
            )
            mask_stage(b, hp, xin[b]["kp1"], e1)
            return e1

        def mod_stage(b, hp, e1):
            """round-1 tail: pv + normalize + PE-transpose + add qh -> qn_pair.

            This transpose is on the s2 latency chain, so it stays on TensorE
            instead of the ~1.2us-per-op DMA xbar path."""
            aug_t = pair_aug[(b, hp)]
            dl = pv_stage(
                b, hp, e1,
                lambda g, jt: aug_t[:, jt, g * AUGW : (g + 1) * AUGW],
            )
            pst = psT.tile([128, 512], bf, tag="tail", name="pst")
            for it in range(4):
                nc.tensor.transpose(
                    pst[:, it * 128 : (it + 1) * 128], dl[:, it, :], ident
                )
            qn_pair = small.tile([128, 512], bf, tag="qnp")
            nc.vector.tensor_add(qn_pair, pst, st[b]["qh"][:, hp, :])
            return qn_pair

        def s2_stage(b, hp, qn_pair):
            e2 = ework.tile([128, 4, 2, L], bf, tag="e")
            kh_sb = st[b]["kh"]
            score_stage(
                b,
                hp,
                lambda g, jt: kh_sb[g * 64 : g * 64 + 64, hp, jt * 128 : (jt + 1) * 128],
                lambda g: qn_pair[g * 64 : g * 64 + 64, :],
                e2,
            )
            mask_stage(b, hp, xin[b]["kp2"], e2)
            return e2

        def av_stage(b, hp, e2):
            """round-2 tail: av + normalize + transpose -> att[:, hp, :].

            Normally via DMA xbar (off the engines); the LAST duo's pairs use
            TensorE transposes instead -- their repack gates the tail output
            projection, and the xbar path costs ~1.2us per tile plus sync-
            queue waits right when the tail needs att complete."""
            vaug_sb = st[b]["vaug"]
            dl = pv_stage(
                b, hp, e2,
                lambda g, jt: vaug_sb[:, jt, (2 * hp + g) * AUGW : (2 * hp + g + 1) * AUGW],
            )
            if (b == 1 and hp >= 6) or (b == 0 and hp == 7):
                pst = psT.tile([128, 512], bf, tag="tail", name="pst")
                for it in range(4):
                    nc.tensor.transpose(
                        pst[:, it * 128 : (it + 1) * 128], dl[:, it, :], ident
                    )
                nc.vector.tensor_copy(out=st[b]["att"][:, hp, :], in_=pst)
            else:
                for it in range(4):
                    nc.sync.dma_start_transpose(
                        out=st[b]["att"][:, hp, it * 128 : (it + 1) * 128],
                        in_=dl[:, it, :],
                    )

        # ---- stage-stream runner ----
        def do_s1(p):
            p["e1"] = s1_stage(p["b"], p["hp"])

        def do_mod(p):
            p["qn"] = mod_stage(p["b"], p["hp"], p["e1"])

        def do_s2(p):
            p["e2"] = s2_stage(p["b"], p["hp"], p["qn"])

        def do_av(p):
            av_stage(p["b"], p["hp"], p["e2"])

        def run_duos(duos, fillers, preduo_hooks):
            # Stage order skewed so score matmuls (ScalarE's exp feed) land
            # every other slot instead of bunching: pb's av carries into the
            # NEXT duo's slot 1, keeping the exp chain gap-free.
            carry = [None]

            def slots(pa, pb):
                return (
                    (do_s1, pa), (do_av, carry[0]),
                    (do_s1, pb), (do_mod, pa),
                    (do_s2, pa), (do_mod, pb),
                    (do_s2, pb), (do_av, pa),
                )

            for di, (pa, pb) in enumerate(duos):
                for hook in preduo_hooks.get(di, ()):
                    hook()
                for fn, p in slots(pa, pb):
                    if p is not None:
                        fn(p)
                    if fillers:
                        t = fillers.pop(0)
                        if t is not None:
                            t()
                carry[0] = pb
            if carry[0] is not None:
                do_av(carry[0])
            while fillers:
                t = fillers.pop(0)
                if t is not None:
                    t()

        # ---- schedule ----
        # prologue: batch 0's three projections, dense PE while inputs land.
        # qh first (so qt(b1)'s WAR clears early), then kh, then vaug.
        # Weight stream on Sync; activation bulk loads on DVE's queue
        # (parallel DMA queues halve the prologue's DMA critical path);
        # per-pair imt/aug + keep-masks on GpSimd's queue.
        load_kp(0, "kp1")
        load_kp(0, "kp2")
        # imt/aug for duo0+duo1's pairs (hooks cover duo2 onward)
        for p_ in ((0, 0), (0, 1), (0, 2), (1, 0)):
            load_pair(*p_)
        kt0 = inp.tile([128, 8, L], bf, tag="kt", name="kt_sb")
        xin[0]["kt"] = kt0
        vt0 = inp.tile([128, 8, L], bf, tag="vt", name="vt_sb")
        xin[0]["vt"] = vt0
        for ot in range(8):
            if ot < 2:
                load_weight_half("wk", wk, ot)
                load_x_half(kt0, kt, 0, ot)
            elif ot < 4:
                load_weight_half("wv", wv, ot - 2)
                load_x_half(vt0, vt, 0, ot - 2)
            proj_qk_group(0, "wq", "qh", ot)
        load_bulk(1, "qt")  # WAR on b0's qh groups just cleared
        load_kp(1, "kp1")
        for ot in range(8):
            proj_qk_group(0, "wk", "kh", ot)
        load_bulk(1, "kt")
        load_kp(1, "kp2")
        for jt in range(4):
            for oh in range(2):
                vaug_group(0, jt, oh)
        load_vt_jt(1)

        P = {}
        for b in range(2):
            for hp in range(8):
                P[(b, hp)] = {"b": b, "hp": hp}

        duos = [
            (P[(0, 0)], P[(0, 1)]),
            (P[(0, 2)], P[(1, 0)]),
            (P[(0, 3)], P[(1, 1)]),
            (P[(0, 4)], P[(1, 2)]),
            (P[(0, 5)], P[(1, 3)]),
            (P[(0, 6)], P[(0, 7)]),
            (P[(1, 4)], P[(1, 5)]),
            (P[(1, 6)], P[(1, 7)]),
        ]

        def halves(fn, *args):
            state = {}
            return [
                (lambda part=part, state=state: fn(*args, part, state))
                for part in range(2)
            ]

        fillers = []
        # duo0: b1's first qh/kh groups + first half of vaug-oh0
        fillers += halves(proj_qk_part, 1, "wq", "qh", 0)
        fillers += halves(proj_qk_part, 1, "wk", "kh", 0)
        fillers += halves(vaug_part, 1, 0, 0)
        fillers += halves(vaug_part, 1, 1, 0)
        # duo1: rest of vaug-oh0 (b1p0's av needs all jt), then ot1
        fillers += halves(vaug_part, 1, 2, 0)
        fillers += halves(vaug_part, 1, 3, 0)
        fillers += halves(proj_qk_part, 1, "wq", "qh", 1)
        fillers += halves(proj_qk_part, 1, "wk", "kh", 1)
        # duo2-4: qh/kh ot2..7
        for ot in range(2, 8):
            fillers += halves(proj_qk_part, 1, "wq", "qh", ot)
            fillers += halves(proj_qk_part, 1, "wk", "kh", ot)
        # duo5: vaug-oh1 (b1p4's av in duo6 needs it)
        for jt in range(4):
            fillers += halves(vaug_part, 1, jt, 1)
        # duo6: b0's output projection (its p7 av lands at duo6 slot 1 via
        # the skew, PE-transposed, so kc4-7 halves are safe from slot 2 on);
        # g3's part1 spills to duo7 slot 0
        fillers.append(None)
        g3_state = {}
        for gi, g in enumerate(((0, 0), (0, 1), (1, 0), (1, 1))):
            stt = g3_state if gi == 3 else {}
            for part in range(2):
                if gi == 3 and part == 1:
                    continue
                fillers.append(
                    lambda it=g[0], oh=g[1], part=part, stt=stt: outproj_part(
                        0, it, oh, part, stt
                    )
                )
        # duo7: rest of b0's groups (full-slot fillers), then PRE-START batch
        # 1's first output groups' kc0-3 accumulations (att pairs 0-3 done in
        # duo4; no score needs psA after duo7 slot 6, so holding 2 psA bufs
        # into the tail is conflict-free)
        fillers.append(lambda: outproj_part(0, 1, 1, 1, g3_state))
        fillers.append(lambda: outproj_group(0, 2, 0))
        fillers.append(lambda: outproj_group(0, 2, 1))
        fillers.append(lambda: outproj_group(0, 3, 0))
        fillers.append(lambda: outproj_group(0, 3, 1))
        b1_pre = {(0, 0): {}, (0, 1): {}}
        for (it, oh), stt in b1_pre.items():
            fillers.append(
                lambda it=it, oh=oh, stt=stt: outproj_part(1, it, oh, 0, stt)
            )

        hooks = {}
        for di in range(len(duos)):
            hs = []
            if di + 2 < len(duos):
                for p in duos[di + 2]:
                    if p is not None:
                        hs.append(
                            lambda b=p["b"], hp=p["hp"]: load_pair(b, hp)
                        )
            if di == 2:
                # wm load deferred off the prologue's saturated DMA window;
                # Sync is quiet by duo2 and the deadline is duo6's outproj
                hs.append(lambda: [load_weight_half("wm", wm, h) for h in range(2)])
            hooks[di] = hs

        run_duos(duos, fillers, hooks)

        # tail: batch 1's output projection (pre-started groups finish first)
        for it in range(4):
            for oh in range(2):
                if (it, oh) in b1_pre:
                    outproj_part(1, it, oh, 1, b1_pre[(it, oh)])
                else:
                    outproj_group(1, it, oh)

    nc.compile()
    return nc


def _get_nc():
    if "nc" not in _CACHE:
        _CACHE["nc"] = _build_nc()
    return _CACHE["nc"]


def _prep_inputs(v, k, q, img_abs, Wv, Wk, Wq, Wm, abs_mask, mask):
    import ml_dtypes

    bf16 = ml_dtypes.bfloat16
    f32 = np.float32

    def swz(x, nt):  # [B, nt*128, F] -> [B, 128, nt*F] partition-contiguous
        b, r, f = x.shape
        return np.ascontiguousarray(
            x.reshape(b, nt, 128, f).transpose(0, 2, 1, 3).reshape(b, 128, nt * f)
        )

    def t_bf(x):  # [B, L, HS] -> [B, 128, 8*L] bf16 swizzled
        xt = np.swapaxes(np.asarray(x, f32), 1, 2)
        return swz(xt, 8).astype(bf16)

    qt = t_bf(q)
    ktr = t_bf(k)
    vtr = t_bf(v)
    imt = t_bf(img_abs)

    img = np.asarray(img_abs, f32)
    augf = np.empty((B, L, H * AUGW), f32)
    augf.reshape(B, L, H, AUGW)[..., :64] = img.reshape(B, L, H, 64)
    augf.reshape(B, L, H, AUGW)[..., 64] = 1.0
    augv = swz(augf, 4).astype(bf16)

    def keepT(m):  # [B, 1, L, L] bool -> (1-m)^T swizzled bf16
        kf = 1.0 - np.asarray(m, f32)[:, 0]
        return swz(np.swapaxes(kf, 1, 2), 4).astype(bf16)

    kp1 = keepT(abs_mask)
    kp2 = keepT(mask)

    def wT(w):
        wt = np.asarray(w, f32).T  # [i, o]
        return swz(wt[None], 8)[0].astype(bf16)

    wqs, wks, wvs, wms = wT(Wq), wT(Wk), wT(Wv), wT(Wm)
    ident = np.eye(128, dtype=bf16)

    in_maps = []
    for c in range(NCORES):
        s = slice(c * BPC, (c + 1) * BPC)
        in_maps.append(
            {
                "qt": qt[s],
                "kt": ktr[s],
                "vt": vtr[s],
                "imt": imt[s],
                "aug": augv[s],
                "kp1": kp1[s],
                "kp2": kp2[s],
                "wq": wqs,
                "wk": wks,
                "wv": wvs,
                "wm": wms,
                "idt": ident,
            }
        )
    return in_maps


def kernel(v, k, q, img_abs, Wv, Wk, Wq, Wm, abs_mask, mask, _trace=False):
    _ensure_concourse()
    from concourse.bass_utils import run_bass_kernel_spmd

    in_maps = _prep_inputs(v, k, q, img_abs, Wv, Wk, Wq, Wm, abs_mask, mask)
    nc = _get_nc()
    res = run_bass_kernel_spmd(nc, in_maps, core_ids=list(range(NCORES)), trace=_trace)
    outp = np.concatenate([res.results[i]["out"] for i in range(NCORES)], axis=0)
    outp = np.asarray(outp, np.float32)  # device stores bf16; upcast on host
    if _trace:
        _CACHE["last_result"] = res
    return outp


# revision 39
# speedup vs baseline: 1.1854x; 1.0400x over previous
"""Trainium2 Bass kernel for nn_ABS_MHAtt (masked two-round multi-head attention).

Strategy: pure data-parallel over batch (B=16 -> 2 batches per NeuronCore, 8 cores,
no collectives). Host-side preprocessing (inside kernel()) pre-transposes
activations/weights into the [contraction, free] layouts the TensorEngine wants and
pre-converts everything to bf16, so the device kernel does zero layout conversion.

Per-core device kernel (per batch):
  - qhT/khT projections in transposed form [o, i]; v projected in natural form [j, o]
    directly into an "augmented" layout with a ones column per head (the ones column
    makes the PV/AV matmul also produce the softmax row-sum).
  - Per head: scores computed transposed [j, i] (contraction over d=64, head pairs
    row-tiled onto the two PE array halves), exp on ScalarE, masking by multiplying
    with (1-mask)^T (split across VectorE and GpSimdE), PV/AV with E as the
    stationary operand, one fused broadcast-multiply normalize per head, and DMA
    xbar transposes (not TensorE) to repack [i, d] tiles back to [d, i].

v3 scheduling: single interleaved stage stream. Prologue = batch 0's three
projections (dense PE while inputs land). Then 8 "duos" of attention pairs mixing
BOTH batches -- (b0p0,b0p1), (b0p2,b1p0), ..., (b1p6,b1p7) -- each duo's 8 stage
slots followed by one ~0.9us filler thunk: batch 1's projections fill the early
duos, batch 0's output projection fills the late duos, so the PE never idles long
enough for the HAM clock gate to re-throttle (attention alone is ScalarE-exp-bound
at ~8.6us/pair vs ~5.1us of PE work). imt/aug are streamed per-pair through small
rotating pools (2-duo lookahead) so both batches' working sets fit in SBUF.
ScalarE runs ONLY exp activations (plus prologue-phase evacuations and tail
stores); bulk loads ride Sync, per-pair loads ride GpSimd.
"""

import os
import sys

import numpy as np


def _ensure_concourse():
    try:
        import concourse.bass  # noqa: F401
        return
    except Exception:
        pass
    for p in ("/opt/trn_rl_repo", "/root/.axon_site/_ro/trn_rl_repo"):
        if os.path.isdir(p) and p not in sys.path:
            sys.path.insert(0, p)
            try:
                import concourse.bass  # noqa: F401
                return
            except Exception:
                sys.path.remove(p)
    raise ImportError("cannot import concourse (bass)")


B, L, HS = 16, 512, 1024
H, D = 16, 64
NCORES = 8
BPC = B // NCORES  # batches per core
SCALE = 1.0 / 8.0  # 1/sqrt(D)
AUGW = 65  # per-head augmented width (D + ones column)

_CACHE = {}


def _build_nc():
    _ensure_concourse()
    import concourse.bass as bass  # noqa: F401
    import concourse.mybir as mybir
    import concourse.tile as tile
    from concourse import bacc
    from contextlib import ExitStack

    bf = mybir.dt.bfloat16
    f32 = mybir.dt.float32
    Exp = mybir.ActivationFunctionType.Exp

    nc = bacc.Bacc()

    # all inputs host-preswizzled to [128, free] per-partition-contiguous
    # layouts so every load is one cheap 2D DMA
    qt = nc.declare_dram_parameter("qt", [BPC, 128, 8 * L], bf, isOutput=False)
    kt = nc.declare_dram_parameter("kt", [BPC, 128, 8 * L], bf, isOutput=False)
    vt = nc.declare_dram_parameter("vt", [BPC, 128, 8 * L], bf, isOutput=False)
    imt = nc.declare_dram_parameter("imt", [BPC, 128, 8 * L], bf, isOutput=False)
    aug = nc.declare_dram_parameter(
        "aug", [BPC, 128, 4 * H * AUGW], bf, isOutput=False
    )
    kp1 = nc.declare_dram_parameter("kp1", [BPC, 128, 4 * L], bf, isOutput=False)
    kp2 = nc.declare_dram_parameter("kp2", [BPC, 128, 4 * L], bf, isOutput=False)
    wq = nc.declare_dram_parameter("wq", [128, 8 * HS], bf, isOutput=False)
    wk = nc.declare_dram_parameter("wk", [128, 8 * HS], bf, isOutput=False)
    wv = nc.declare_dram_parameter("wv", [128, 8 * HS], bf, isOutput=False)
    wm = nc.declare_dram_parameter("wm", [128, 8 * HS], bf, isOutput=False)
    idt = nc.declare_dram_parameter("idt", [128, 128], bf, isOutput=False)
    out = nc.declare_dram_parameter("out", [BPC, L, HS], bf, isOutput=True)

    with ExitStack() as ctx:
        tc = ctx.enter_context(tile.TileContext(nc))
        consts = ctx.enter_context(tc.tile_pool(name="consts", bufs=1))
        inp = ctx.enter_context(tc.tile_pool(name="inp", bufs=1))
        rot = ctx.enter_context(tc.tile_pool(name="rot", bufs=6))
        proj = ctx.enter_context(tc.tile_pool(name="proj", bufs=2))
        ework = ctx.enter_context(tc.tile_pool(name="ework", bufs=2))
        small = ctx.enter_context(tc.tile_pool(name="small", bufs=3))
        evac = ctx.enter_context(tc.tile_pool(name="evac", bufs=4))
        psA = ctx.enter_context(tc.tile_pool(name="psA", bufs=3, space="PSUM"))
        psT = ctx.enter_context(tc.tile_pool(name="psT", bufs=2, space="PSUM"))

        w_sb = {}

        def load_weight_half(name, wext, half, tag=None, eng=None):
            tag = tag or name
            if name not in w_sb:
                t = consts.tile([128, 8, HS], bf, tag=tag, name=name + "_sb")
                w_sb[name] = t
            t = w_sb[name]
            (eng or nc.sync).dma_start(
                out=t[:, half * 4 : (half + 1) * 4, :],
                in_=wext[:, half * 4 * HS : (half + 1) * 4 * HS],
            )

        # ---- bulk per-batch input tiles (qt/kt/vt shared between batches:
        # batch 1's load WAR-waits on batch 0's last projection reader) ----
        xin = {0: {}, 1: {}}

        def load_x_half(t, ext, b, half):
            # batch 0's bulk loads ride the ScalarE HWDGE queue (exp-idle
            # during the prologue), parallel to the weight stream on Sync;
            # batch 1's ride GpSimd so they don't crowd b0's critical path
            eng = nc.scalar if b == 0 else nc.gpsimd
            eng.dma_start(
                out=t[:, half * 4 : (half + 1) * 4, :],
                in_=ext[b][:, half * 4 * L : (half + 1) * 4 * L],
            )

        def load_bulk(b, which):
            ext = {"qt": qt, "kt": kt, "vt": vt}[which]
            t = inp.tile([128, 8, L], bf, tag=which, name=which + "_sb")
            for half in range(2):
                load_x_half(t, ext, b, half)
            xin[b][which] = t

        def load_vt_jt(b):
            # jt-sliced vt load: 4 DMAs of [128, 8, 128] so the first vaug
            # group's slice lands after ~0.25MB instead of 1MB
            t = inp.tile([128, 8, L], bf, tag="vt", name="vt_sb")
            ext = vt[b].rearrange("p (c x) -> p c x", x=L)
            eng = nc.scalar if b == 0 else nc.gpsimd
            for jt in range(4):
                eng.dma_start(
                    out=t[:, :, jt * 128 : (jt + 1) * 128],
                    in_=ext[:, :, jt * 128 : (jt + 1) * 128],
                )
            xin[b]["vt"] = t

        def load_kp(b, which):
            ext = {"kp1": kp1, "kp2": kp2}[which]
            t = inp.tile([128, 4, L], bf, tag=f"{which}_{b}", name=which + "_sb")
            nc.gpsimd.dma_start(out=t, in_=ext[b])
            xin[b][which] = t

        # ---- per-pair rotating imt/aug tiles (2-duo lookahead) ----
        pair_imt = {}
        pair_aug = {}

        def load_pair(b, hp):
            ti = rot.tile([128, L], bf, tag="imt", name="imt_p")
            nc.gpsimd.dma_start(
                out=ti, in_=imt[b][:, hp * L : (hp + 1) * L]
            )
            pair_imt[(b, hp)] = ti
            ta = rot.tile([128, 4, 2 * AUGW], bf, tag="aug", name="aug_p")
            nc.gpsimd.dma_start(
                out=ta,
                in_=aug[b].rearrange("p (jt x) -> p jt x", x=H * AUGW)[
                    :, :, 2 * hp * AUGW : (2 * hp + 2) * AUGW
                ],
            )
            pair_aug[(b, hp)] = ta

        # ---- startup: wq/qt in fine-grained interleaved chunks so the first
        # projection matmuls can start after ~400KB instead of 3MB ----
        ident = consts.tile([128, 128], bf, tag="ident")
        nc.gpsimd.dma_start(out=ident, in_=idt[:, :])
        xin[0]["qt"] = inp.tile([128, 8, L], bf, tag="qt", name="qt_sb")
        wq_t = consts.tile([128, 8, HS], bf, tag="wq", name="wq_sb")
        w_sb["wq"] = wq_t
        for c in range(4):
            nc.sync.dma_start(
                out=wq_t[:, 2 * c : 2 * c + 2, :],
                in_=wq[:, 2 * c * HS : (2 * c + 2) * HS],
            )
            nc.scalar.dma_start(
                out=xin[0]["qt"][:, 2 * c : 2 * c + 2, :],
                in_=qt[0][:, 2 * c * L : (2 * c + 2) * L],
            )
        # HAM warm-up: ~3.5us of back-to-back dummy transposes (gated only on
        # the tiny ident load) promote the PE clock gate to 2.4 GHz during the
        # DMA-bound startup, instead of ramping mid-prologue
        for w in range(64):
            wps = psT.tile([128, 128], bf, tag="tail", name="warm")
            nc.tensor.transpose(wps, ident, ident)

        # ---- per-batch working tiles ----
        st = {}
        for b in range(BPC):
            st[b] = {
                "qh": proj.tile([128, 8, L], bf, tag="qh", name="qh_sb"),
                "kh": proj.tile([128, 8, L], bf, tag="kh", name="kh_sb"),
                "vaug": proj.tile(
                    [128, 4, H * AUGW], bf, tag="vaug", name="vaug_sb"
                ),
                "att": proj.tile([128, 8, L], bf, tag="att", name="att_sb"),
            }

        # ---- projection groups (one PSUM group each; interleavable thunks) ----
        def proj_qk_part(b, wname, dstname, ot, part, state):
            """half of a projection PSUM group (~0.9us of PE work)."""
            wt = w_sb[wname]
            xsb = xin[b]["qt" if wname == "wq" else "kt"]
            dst = st[b][dstname]
            if part == 0:
                state["ps"] = psA.tile([128, 512], f32, tag="psA", name="ps_pj")
            ps = state["ps"]
            for kc in range(4 * part, 4 * part + 4):
                nc.tensor.matmul(
                    ps,
                    wt[:, kc, ot * 128 : (ot + 1) * 128],
                    xsb[:, kc, :],
                    start=(kc == 0),
                    stop=(kc == 7),
                )
            if part == 1:
                if b == 0:
                    # b0's evacs ride ScalarE (exp-idle during the prologue);
                    # b1's ride DVE (GpSimd has no PSUM port)
                    nc.scalar.copy(out=dst[:, ot, :], in_=ps)
                else:
                    nc.vector.tensor_copy(out=dst[:, ot, :], in_=ps)

        def proj_qk_group(b, wname, dstname, ot):
            state = {}
            proj_qk_part(b, wname, dstname, ot, 0, state)
            proj_qk_part(b, wname, dstname, ot, 1, state)

        def vaug_part(b, jt, oh, part, state):
            """half of a v-projection group into the augmented layout."""
            vaug_sb = st[b]["vaug"]
            vt_sb = xin[b]["vt"]
            if part == 0 and oh == 0:
                nc.vector.memset(
                    vaug_sb[:, jt, :].rearrange("p (h x) -> p h x", x=AUGW)[
                        :, :, 64
                    ],
                    1.0,
                )
            if part == 0:
                state["ps"] = psA.tile([128, 512], f32, tag="psA", name="ps_pj")
            ps = state["ps"]
            for kc in range(4 * part, 4 * part + 4):
                nc.tensor.matmul(
                    ps,
                    vt_sb[:, kc, jt * 128 : (jt + 1) * 128],
                    w_sb["wv"][:, kc, oh * 512 : (oh + 1) * 512],
                    start=(kc == 0),
                    stop=(kc == 7),
                )
            if part == 1:
                dst_ap = vaug_sb[
                    :, jt, oh * 8 * AUGW : (oh + 1) * 8 * AUGW
                ].rearrange("p (h x) -> p h x", x=AUGW)[:, :, 0:64]
                nc.vector.tensor_copy(
                    out=dst_ap, in_=ps.rearrange("p (h x) -> p h x", x=64)
                )

        def vaug_group(b, jt, oh):
            state = {}
            vaug_part(b, jt, oh, 0, state)
            vaug_part(b, jt, oh, 1, state)

        def outproj_part(b, it, oh, part, state):
            """half of an output-projection PSUM group (~0.9us of PE work)."""
            att_sb = st[b]["att"]
            if part == 0:
                state["ps"] = psA.tile([128, 512], f32, tag="psA", name="ps_pj")
            ps = state["ps"]
            for kc in range(4 * part, 4 * part + 4):
                nc.tensor.matmul(
                    ps,
                    att_sb[:, kc, it * 128 : (it + 1) * 128],
                    w_sb["wm"][:, kc, oh * 512 : (oh + 1) * 512],
                    start=(kc == 0),
                    stop=(kc == 7),
                )
            if part == 1:
                ob = evac.tile([128, 512], bf, tag="ob")
                g = 2 * it + oh
                if b == 1:
                    # tail: exp is done -- alternate evacs across ScalarE and
                    # DVE, and stores across three DMA queues, so the 8
                    # groups' drains run as parallel chains instead of one
                    if g % 2 == 0:
                        nc.scalar.copy(out=ob, in_=ps)
                    else:
                        nc.vector.tensor_copy(out=ob, in_=ps)
                    eng = (nc.scalar, nc.gpsimd, nc.sync)[g % 3]
                else:
                    nc.vector.tensor_copy(out=ob, in_=ps)
                    # keep Sync transpose-only during the region: stores
                    # there convoy with the xbar transposes and head-of-line
                    # block DVE via the evac-buffer WAR
                    eng = nc.gpsimd
                eng.dma_start(
                    out=out[
                        b, it * 128 : (it + 1) * 128, oh * 512 : (oh + 1) * 512
                    ],
                    in_=ob,
                )

        def outproj_group(b, it, oh):
            state = {}
            outproj_part(b, it, oh, 0, state)
            outproj_part(b, it, oh, 1, state)

        # ---- attention stages ----
        def score_stage(b, hp, lhs_fn, rhs_fn, etile):
            """s^T [j,i] for both heads of pair hp + exp into etile.

            The two heads' matmuls use lhsT base partitions 0 / 64, so they
            run concurrently on the two row-halves of the PE array (outputs
            land in different PSUM banks)."""
            for jt in range(4):
                ps = psA.tile([128, 1024], f32, tag="psA", name="ps_sc")
                for g in range(2):
                    nc.tensor.matmul(
                        ps[:, g * 512 : (g + 1) * 512],
                        lhs_fn(g, jt),
                        rhs_fn(g),
                        start=True,
                        stop=True,
                    )
                nc.scalar.activation(
                    out=etile[:, jt],
                    in_=ps.rearrange("p (g x) -> p g x", x=512),
                    func=Exp,
                    scale=SCALE,
                )

        def mask_stage(b, hp, kp_sb, etile):
            # in-place mask multiply, one fused op per jt covering both heads
            # (mask row broadcast across the head dim via a stride-0 AP).
            # All on DVE: a GpSimd-offloaded op costs ~2.2us on the
            # exp->mask->pv critical chain and stalls the in-order PE queue.
            for jt in range(4):
                kpb = kp_sb[:, jt, :].unsqueeze(1).broadcast_to([128, 2, L])
                nc.vector.tensor_mul(etile[:, jt], etile[:, jt], kpb)

        def pv_stage(b, hp, emtile, rhs_fn):
            """pv natural [i, 4*65] per head -> normalized dl pair [128,4,128].

            One fused broadcast-multiply per head turns the raw PSUM pv tile
            into the normalized bf16 dl tile (recip row-sums broadcast along
            d via a stride-0 AP)."""
            dl = small.tile([128, 4, 128], bf, tag="dl")
            for g in range(2):
                pspv = psT.tile([128, 4, AUGW], f32, tag="tail")
                for it in range(4):
                    for jt in range(4):
                        nc.tensor.matmul(
                            pspv[:, it, :],
                            emtile[:, jt, g, it * 128 : (it + 1) * 128],
                            rhs_fn(g, jt),
                            start=(jt == 0),
                            stop=(jt == 3),
                        )
                r1 = small.tile([128, 4], f32, tag="r1")
                nc.vector.reciprocal(r1, pspv[:, :, 64])
                nc.vector.tensor_mul(
                    dl[:, :, g * 64 : (g + 1) * 64],
                    pspv[:, :, 0:64],
                    r1.unsqueeze(-1).broadcast_to([128, 4, 64]),
                )
            return dl

        def s1_stage(b, hp):
            e1 = ework.tile([128, 4, 2, L], bf, tag="e")
            imt_t = pair_imt[(b, hp)]
            qh_sb = st[b]["qh"]
            score_stage(
                b,
                hp,
                lambda g, jt: imt_t[g * 64 : g * 64 + 64, jt * 128 : (jt + 1) * 128],
                lambda g: qh_sb[g * 64 : g * 64 + 64, hp, :],
                e1,
            )
            mask_stage(b, hp, xin[b]["kp1"], e1)
            return e1

        def mod_stage(b, hp, e1):
            """round-1 tail: pv + normalize + PE-transpose + add qh -> qn_pair.

            This transpose is on the s2 latency chain, so it stays on TensorE
            instead of the ~1.2us-per-op DMA xbar path."""
            aug_t = pair_aug[(b, hp)]
            dl = pv_stage(
                b, hp, e1,
                lambda g, jt: aug_t[:, jt, g * AUGW : (g + 1) * AUGW],
            )
            pst = psT.tile([128, 512], bf, tag="tail", name="pst")
            for it in range(4):
                nc.tensor.transpose(
                    pst[:, it * 128 : (it + 1) * 128], dl[:, it, :], ident
                )
            qn_pair = small.tile([128, 512], bf, tag="qnp")
            nc.vector.tensor_add(qn_pair, pst, st[b]["qh"][:, hp, :])
            return qn_pair

        def s2_stage(b, hp, qn_pair):
            e2 = ework.tile([128, 4, 2, L], bf, tag="e")
            kh_sb = st[b]["kh"]
            score_stage(
                b,
                hp,
                lambda g, jt: kh_sb[g * 64 : g * 64 + 64, hp, jt * 128 : (jt + 1) * 128],
                lambda g: qn_pair[g * 64 : g * 64 + 64, :],
                e2,
            )
            mask_stage(b, hp, xin[b]["kp2"], e2)
            return e2

        def av_stage(b, hp, e2):
            """round-2 tail: av + normalize + transpose -> att[:, hp, :].

            Normally via DMA xbar (off the engines); the LAST duo's pairs use
            TensorE transposes instead -- their repack gates the tail output
            projection, and the xbar path costs ~1.2us per tile plus sync-
            queue waits right when the tail needs att complete."""
            vaug_sb = st[b]["vaug"]
            dl = pv_stage(
                b, hp, e2,
                lambda g, jt: vaug_sb[:, jt, (2 * hp + g) * AUGW : (2 * hp + g + 1) * AUGW],
            )
            if (b == 1 and hp >= 6) or (b == 0 and hp == 7):
                pst = psT.tile([128, 512], bf, tag="tail", name="pst")
                for it in range(4):
                    nc.tensor.transpose(
                        pst[:, it * 128 : (it + 1) * 128], dl[:, it, :], ident
                    )
                nc.vector.tensor_copy(out=st[b]["att"][:, hp, :], in_=pst)
            else:
                for it in range(4):
                    nc.sync.dma_start_transpose(
                        out=st[b]["att"][:, hp, it * 128 : (it + 1) * 128],
                        in_=dl[:, it, :],
                    )

        # ---- stage-stream runner ----
        def do_s1(p):
            p["e1"] = s1_stage(p["b"], p["hp"])

        def do_mod(p):
            p["qn"] = mod_stage(p["b"], p["hp"], p["e1"])

        def do_s2(p):
            p["e2"] = s2_stage(p["b"], p["hp"], p["qn"])

        def do_av(p):
            av_stage(p["b"], p["hp"], p["e2"])

        def run_duos(duos, fillers, preduo_hooks):
            # Stage order skewed so score matmuls (ScalarE's exp feed) land
            # every other slot instead of bunching: pb's av carries into the
            # NEXT duo's slot 1, keeping the exp chain gap-free.
            carry = [None]

            def slots(pa, pb):
                return (
                    (do_s1, pa), (do_av, carry[0]),
                    (do_s1, pb), (do_mod, pa),
                    (do_s2, pa), (do_mod, pb),
                    (do_s2, pb), (do_av, pa),
                )

            for di, (pa, pb) in enumerate(duos):
                for hook in preduo_hooks.get(di, ()):
                    hook()
                for fn, p in slots(pa, pb):
                    if p is not None:
                        fn(p)
                    if fillers:
                        t = fillers.pop(0)
                        if t is not None:
                            t()
                carry[0] = pb
            if carry[0] is not None:
                do_av(carry[0])
            while fillers:
                t = fillers.pop(0)
                if t is not None:
                    t()

        # ---- schedule ----
        # prologue: batch 0's three projections, dense PE while inputs land.
        # qh first (so qt(b1)'s WAR clears early), then kh, then vaug.
        # Weight stream on Sync; activation bulk loads on DVE's queue
        # (parallel DMA queues halve the prologue's DMA critical path);
        # per-pair imt/aug + keep-masks on GpSimd's queue.
        load_kp(0, "kp1")
        load_kp(0, "kp2")
        # imt/aug for duo0+duo1's pairs (hooks cover duo2 onward)
        for p_ in ((0, 0), (0, 1), (0, 2), (1, 0)):
            load_pair(*p_)
        kt0 = inp.tile([128, 8, L], bf, tag="kt", name="kt_sb")
        xin[0]["kt"] = kt0
        vt0 = inp.tile([128, 8, L], bf, tag="vt", name="vt_sb")
        xin[0]["vt"] = vt0
        for ot in range(8):
            if ot < 2:
                load_weight_half("wk", wk, ot)
                load_x_half(kt0, kt, 0, ot)
            elif ot < 4:
                load_weight_half("wv", wv, ot - 2)
                load_x_half(vt0, vt, 0, ot - 2)
            proj_qk_group(0, "wq", "qh", ot)
        load_bulk(1, "qt")  # WAR on b0's qh groups just cleared
        load_kp(1, "kp1")
        for ot in range(8):
            proj_qk_group(0, "wk", "kh", ot)
        load_bulk(1, "kt")
        load_kp(1, "kp2")
        for jt in range(4):
            for oh in range(2):
                vaug_group(0, jt, oh)
        load_vt_jt(1)

        P = {}
        for b in range(2):
            for hp in range(8):
                P[(b, hp)] = {"b": b, "hp": hp}

        duos = [
            (P[(0, 0)], P[(0, 1)]),
            (P[(0, 2)], P[(1, 0)]),
            (P[(0, 3)], P[(1, 1)]),
            (P[(0, 4)], P[(1, 2)]),
            (P[(0, 5)], P[(1, 3)]),
            (P[(0, 6)], P[(0, 7)]),
            (P[(1, 4)], P[(1, 5)]),
            (P[(1, 6)], P[(1, 7)]),
        ]

        def halves(fn, *args):
            state = {}
            return [
                (lambda part=part, state=state: fn(*args, part, state))
                for part in range(2)
            ]

        fillers = []
        # duo0: b1's first qh/kh groups + first half of vaug-oh0
        fillers += halves(proj_qk_part, 1, "wq", "qh", 0)
        fillers += halves(proj_qk_part, 1, "wk", "kh", 0)
        fillers += halves(vaug_part, 1, 0, 0)
        fillers += halves(vaug_part, 1, 1, 0)
        # duo1: rest of vaug-oh0 (b1p0's av needs all jt), then ot1
        fillers += halves(vaug_part, 1, 2, 0)
        fillers += halves(vaug_part, 1, 3, 0)
        fillers += halves(proj_qk_part, 1, "wq", "qh", 1)
        fillers += halves(proj_qk_part, 1, "wk", "kh", 1)
        # duo2-4: qh/kh ot2..7
        for ot in range(2, 8):
            fillers += halves(proj_qk_part, 1, "wq", "qh", ot)
            fillers += halves(proj_qk_part, 1, "wk", "kh", ot)
        # duo5: vaug-oh1 (b1p4's av in duo6 needs it)
        for jt in range(4):
            fillers += halves(vaug_part, 1, jt, 1)
        # duo6: b0's output projection (its p7 av lands at duo6 slot 1 via
        # the skew, PE-transposed, so kc4-7 halves are safe from slot 2 on);
        # g3's part1 spills to duo7 slot 0
        fillers.append(None)
        g3_state = {}
        for gi, g in enumerate(((0, 0), (0, 1), (1, 0), (1, 1))):
            stt = g3_state if gi == 3 else {}
            for part in range(2):
                if gi == 3 and part == 1:
                    continue
                fillers.append(
                    lambda it=g[0], oh=g[1], part=part, stt=stt: outproj_part(
                        0, it, oh, part, stt
                    )
                )
        # duo7: rest of b0's groups (full-slot fillers), then PRE-START batch
        # 1's first output groups' kc0-3 accumulations (att pairs 0-3 done in
        # duo4; no score needs psA after duo7 slot 6, so holding 2 psA bufs
        # into the tail is conflict-free)
        fillers.append(lambda: outproj_part(0, 1, 1, 1, g3_state))
        fillers.append(lambda: outproj_group(0, 2, 0))
        fillers.append(lambda: outproj_group(0, 2, 1))
        fillers.append(lambda: outproj_group(0, 3, 0))
        fillers.append(lambda: outproj_group(0, 3, 1))
        b1_pre = {(0, 0): {}, (0, 1): {}}
        for (it, oh), stt in b1_pre.items():
            fillers.append(
                lambda it=it, oh=oh, stt=stt: outproj_part(1, it, oh, 0, stt)
            )

        hooks = {}
        for di in range(len(duos)):
            hs = []
            if di + 2 < len(duos):
                for p in duos[di + 2]:
                    if p is not None:
                        hs.append(
                            lambda b=p["b"], hp=p["hp"]: load_pair(b, hp)
                        )
            if di == 2:
                # wm load deferred off the prologue's saturated DMA window;
                # Sync is quiet by duo2 and the deadline is duo6's outproj
                hs.append(
                    lambda: [
                        load_weight_half("wm", wm, h, eng=nc.gpsimd)
                        for h in range(2)
                    ]
                )
            hooks[di] = hs

        run_duos(duos, fillers, hooks)

        # tail: batch 1's output projection (pre-started groups finish first)
        for it in range(4):
            for oh in range(2):
                if (it, oh) in b1_pre:
                    outproj_part(1, it, oh, 1, b1_pre[(it, oh)])
                else:
                    outproj_group(1, it, oh)

    nc.compile()
    return nc


def _get_nc():
    if "nc" not in _CACHE:
        _CACHE["nc"] = _build_nc()
    return _CACHE["nc"]


def _prep_inputs(v, k, q, img_abs, Wv, Wk, Wq, Wm, abs_mask, mask):
    import ml_dtypes

    bf16 = ml_dtypes.bfloat16
    f32 = np.float32

    def swz(x, nt):  # [B, nt*128, F] -> [B, 128, nt*F] partition-contiguous
        b, r, f = x.shape
        return np.ascontiguousarray(
            x.reshape(b, nt, 128, f).transpose(0, 2, 1, 3).reshape(b, 128, nt * f)
        )

    def t_bf(x):  # [B, L, HS] -> [B, 128, 8*L] bf16 swizzled
        xt = np.swapaxes(np.asarray(x, f32), 1, 2)
        return swz(xt, 8).astype(bf16)

    qt = t_bf(q)
    ktr = t_bf(k)
    vtr = t_bf(v)
    imt = t_bf(img_abs)

    img = np.asarray(img_abs, f32)
    augf = np.empty((B, L, H * AUGW), f32)
    augf.reshape(B, L, H, AUGW)[..., :64] = img.reshape(B, L, H, 64)
    augf.reshape(B, L, H, AUGW)[..., 64] = 1.0
    augv = swz(augf, 4).astype(bf16)

    def keepT(m):  # [B, 1, L, L] bool -> (1-m)^T swizzled bf16
        kf = 1.0 - np.asarray(m, f32)[:, 0]
        return swz(np.swapaxes(kf, 1, 2), 4).astype(bf16)

    kp1 = keepT(abs_mask)
    kp2 = keepT(mask)

    def wT(w):
        wt = np.asarray(w, f32).T  # [i, o]
        return swz(wt[None], 8)[0].astype(bf16)

    wqs, wks, wvs, wms = wT(Wq), wT(Wk), wT(Wv), wT(Wm)
    ident = np.eye(128, dtype=bf16)

    in_maps = []
    for c in range(NCORES):
        s = slice(c * BPC, (c + 1) * BPC)
        in_maps.append(
            {
                "qt": qt[s],
                "kt": ktr[s],
                "vt": vtr[s],
                "imt": imt[s],
                "aug": augv[s],
                "kp1": kp1[s],
                "kp2": kp2[s],
                "wq": wqs,
                "wk": wks,
                "wv": wvs,
                "wm": wms,
                "idt": ident,
            }
        )
    return in_maps


def kernel(v, k, q, img_abs, Wv, Wk, Wq, Wm, abs_mask, mask, _trace=False):
    _ensure_concourse()
    from concourse.bass_utils import run_bass_kernel_spmd

    in_maps = _prep_inputs(v, k, q, img_abs, Wv, Wk, Wq, Wm, abs_mask, mask)
    nc = _get_nc()
    res = run_bass_kernel_spmd(nc, in_maps, core_ids=list(range(NCORES)), trace=_trace)
    outp = np.concatenate([res.results[i]["out"] for i in range(NCORES)], axis=0)
    outp = np.asarray(outp, np.float32)  # device stores bf16; upcast on host
    if _trace:
        _CACHE["last_result"] = res
    return outp
